# revision 27
# baseline (speedup 1.0000x reference)
"""Trainium2 Bass kernel for HeavilyCompressedAttention.

Sharding: 16 heads across 8 cores (2 heads/core, tensor-parallel);
compressed-KV path (single shared head) replicated on every core;
out_proj row-parallel with host-side partial sum (bf16 partials).

v2: weight DMAs split per k-group for early P1 start, hN prefetched,
qkT/pT transposes batched through one PSUM tile with single wide
copies, combined per-i mask table, sink handled as rank-1 matmul,
SBUF-only elementwise ops on GpSimd, P4 fused into the P3 loop,
zero-bias fast path.
"""

import os
import sys

import numpy as np
import ml_dtypes

for _p in ("/opt/trn_rl_repo", "/root/.axon_site/_ro/trn_rl_repo"):
    if os.path.isdir(_p) and _p not in sys.path:
        sys.path.insert(0, _p)

from concourse import bacc, mybir  # noqa: E402
import concourse.tile as tile  # noqa: E402
from concourse.bass_utils import run_bass_kernel_spmd  # noqa: E402
from concourse.masks import make_identity  # noqa: E402

F32 = mybir.dt.float32
BF16 = mybir.dt.bfloat16
NPBF = ml_dtypes.bfloat16

S = 2048
HID = 2048
NH = 16
HD = 128
R = 16
C = S // R  # 128
WIN = 128
ROPE = HD // 2  # 64
HALF = ROPE // 2  # 32
EPS = 1e-6
NT = S // 128  # 16 s-tiles
KT = HID // 128  # 16 k-tiles
NCORES = 8
HPC = NH // NCORES  # 2 heads per core
SCALE = 1.0 / float(np.sqrt(HD))
MASKV = -30000.0
KG = 4  # k-tiles per weight-chunk DMA

_CACHE = {}

# ---- const blob column layout (bf16 [128, CB]) ----
_OFF = {}
_cb = 0


def _col(name, n):
    global _cb
    _OFF[name] = (_cb, _cb + n)
    _cb += n


_col("wg0", KG * 769)       # per k: [wqlk_k (512) | wlvc_k (257)]
_col("tA", NT * 64)         # --- SMALL chunk (needed early in P1) ---
_col("tB", NT * 64)
_col("tC", NT * 64)
_col("tD", NT * 64)
_col("qk_pass", 128)
_col("ctA", HALF)           # rows 0:C
_col("ctB", HALF)
_col("ctC", HALF)
_col("ctD", HALF)
_col("ck_pass", ROPE)       # rows 0:C
_col("dwide", 248)
_col("b_qlk", 512)          # row 0
_col("b_lvc", 257)          # row 0
_col("b_kv", 256)           # row 0
_col("sinkkT", HPC)         # [128, HPC] column per head
_col("sinkv", HPC * 128)    # row 0 only used (rank-1 matmul)
for _g in range(1, KT // KG):
    _col(f"wg{_g}", KG * 769)
_col("mloc", NT * 256)      # per-i local mask (used during P1)
_col("wkv", KT * 256)       # --- BIG tail (needed from P2 on) ---
_col("wo", HPC * HID)       # [p, h, HID]  (0.5 folded in)
_col("mcmp", NT * 129)      # per-i [comp 128 | sink 0] mask
CB = _cb
_SMALL = (_OFF["tA"][0], _OFF["sinkv"][1])
_BIG = (_OFF["wkv"][0], CB)


def _build_bass(zero_bias):
    nc = bacc.Bacc("TRN2", target_bir_lowering=False, debug=False,
                   num_devices=NCORES)

    din = {}

    def inp(name, shape, dt):
        din[name] = nc.dram_tensor(name, list(shape), dt, kind="ExternalInput")
        return din[name]

    hT = inp("hT", [NT, 128, KT * 128], BF16)   # [i][hid_p][k*128+c(s)]
    hN = inp("hN", [NT // 2, 128, 2 * HID], BF16)  # [j][s_p][jj*HID+hid]
    cblob = inp("cblob", [128, CB], BF16)

    out_p = nc.dram_tensor("out_p", [NT, 128, HID], BF16,
                           kind="ExternalOutput")

    with tile.TileContext(nc) as tc:
        with (
            tc.tile_pool(name="const", bufs=1) as cst,
            tc.tile_pool(name="persist", bufs=1) as per,
            tc.tile_pool(name="stream", bufs=3) as stm,
            tc.tile_pool(name="scratch", bufs=3) as scr,
            tc.tile_pool(name="stats", bufs=6) as sts,
        ):
            blob = cst.tile([128, CB], BF16, name="c_blob")
            # wg0 + small tables first (P1 start), wg1..3 next;
            # big tail (wkv|wo|mcomb) emitted just before P2
            a, b = _OFF["wg0"]
            nc.sync.dma_start(out=blob[:, a:b], in_=cblob.ap()[:, a:b])
            nc.sync.dma_start(out=blob[:, _SMALL[0]:_SMALL[1]],
                              in_=cblob.ap()[:, _SMALL[0]:_SMALL[1]])
            for g in range(1, KT // KG):
                a, b = _OFF[f"wg{g}"]
                nc.sync.dma_start(out=blob[:, a:b],
                                  in_=cblob.ap()[:, a:b])
            a, b = _OFF["mloc"]
            nc.sync.dma_start(out=blob[:, a:b], in_=cblob.ap()[:, a:b])

            def cv(name):
                a, b = _OFF[name]
                return blob[:, a:b]

            # weight views: wqlk k -> wg{k//KG} cols [(k%KG)*769, +512]
            def wqlk_k(k):
                base = _OFF[f"wg{k // KG}"][0] + (k % KG) * 769
                return blob[:, base:base + 512]

            def wlvc_k(k):
                base = _OFF[f"wg{k // KG}"][0] + (k % KG) * 769 + 512
                return blob[:, base:base + 257]

            wkv_sb = cv("wkv").rearrange("p (k n) -> p k n", k=KT)
            wo_sb = cv("wo").rearrange("p (h n) -> p h n", h=HPC)
            tA_sb = cv("tA").rearrange("p (i n) -> p i n", i=NT)
            tB_sb = cv("tB").rearrange("p (i n) -> p i n", i=NT)
            tC_sb = cv("tC").rearrange("p (i n) -> p i n", i=NT)
            tD_sb = cv("tD").rearrange("p (i n) -> p i n", i=NT)
            qk_pass_sb = cv("qk_pass")
            ctA_sb = cv("ctA")[0:C, :]
            ctB_sb = cv("ctB")[0:C, :]
            ctC_sb = cv("ctC")[0:C, :]
            ctD_sb = cv("ctD")[0:C, :]
            ck_pass_sb = cv("ck_pass")[0:C, :]
            mloc_sb = cv("mloc").rearrange("p (i n) -> p i n", i=NT)
            mcmp_sb = cv("mcmp").rearrange("p (i n) -> p i n", i=NT)
            dwide_sb = cv("dwide")
            b_qlk_sb = cv("b_qlk")[0:1, :]
            b_lvc_sb = cv("b_lvc")[0:1, :]
            b_kv_sb = cv("b_kv")[0:1, :]
            sinkkT_sb = cv("sinkkT")
            sinkv_sb = cv("sinkv")[0:1, :]

            ident_bf = cst.tile([128, 128], BF16)
            make_identity(nc, ident_bf[:])
            ident_f32 = cst.tile([128, 128], F32)
            make_identity(nc, ident_f32[:])
            ones1 = cst.tile([1, 128], BF16)
            nc.vector.memset(ones1[:], 1.0)
            eps_t = cst.tile([128, 1], F32)
            nc.vector.memset(eps_t[:], EPS)

            # ---- persistent activations ----
            lvn = per.tile([128, NT, 256], BF16)    # local v (natural)
            # q0|q1|lk0|lk1 transposed, slot i+1 = tile i; slot0 lk = 0
            qkT = per.tile([128, NT + 1, 512], BF16)
            cwN = per.tile([128, NT], F32)          # compressor scores
            wN = per.tile([128, NT], F32)           # block-softmaxed weights
            entries = per.tile([C, HID], BF16)
            eT = per.tile([128, KT, C], BF16)       # entries^T tiles
            cvn = per.tile([C, 128], BF16)
            ckT_aug = per.tile([128, HPC, 129], BF16)  # ck^T | sink_k col
            mgT = per.tile([128, HPC, S], BF16)     # merged^T = cc+cl

            nc.vector.memset(qkT[:, 0, 256:512], 0.0)
            for h in range(HPC):
                nc.vector.tensor_copy(ckT_aug[:, h, 128:129],
                                      sinkkT_sb[:, h:h + 1])

            # ================= P1: projections + norm/rope =================
            with (
                tc.tile_pool(name="ps_qlk", bufs=2, space="PSUM") as pq,
                tc.tile_pool(name="ps_lvc", bufs=2, space="PSUM") as pl,
                tc.tile_pool(name="ps_tp", bufs=1, space="PSUM") as ptp,
                tc.tile_pool(name="ps_sl", bufs=1, space="PSUM") as psl,
                tc.tile_pool(name="ps_tpl", bufs=1, space="PSUM") as ptpl,
                tc.tile_pool(name="ps_cl", bufs=1, space="PSUM") as pcl,
            ):
                def front_l(i, h):
                    # local-branch scores+mask+softmax for pair (i, h)
                    qT = qkT[:, i + 1, h * 128:(h + 1) * 128]
                    ps_sl = psl.tile([128, 256], F32, tag="sl")
                    nc.tensor.matmul(ps_sl[:], ident_bf[:],
                                     mloc_sb[:, i, :],
                                     start=True, stop=False)
                    nc.tensor.matmul(
                        ps_sl[:], qT,
                        qkT[:, i:i + 2, 256 + h * 128:256 + (h + 1) * 128],
                        start=False, stop=True)
                    p_l = scr.tile([128, 256], BF16, tag="pl", bufs=4)
                    den_l = sts.tile([128, 1], F32)
                    nc.scalar.activation(p_l[:], ps_sl[:],
                                         mybir.ActivationFunctionType.Exp,
                                         scale=SCALE, accum_out=den_l[:])
                    return p_l, den_l

                def back_l(i, h, p_l, den_l):
                    rden = sts.tile([128, 1], F32)
                    nc.vector.reciprocal(rden[:], den_l[:])
                    rdg = scr.tile([128, 128], BF16, tag="rdgl", bufs=3)
                    nc.vector.tensor_scalar_mul(rdg[:], ident_bf[:],
                                                rden[:])
                    ps_tpl = ptpl.tile([128, 256], F32, tag="tpl")
                    for cidx in range(2):
                        nc.tensor.matmul(
                            ps_tpl[:, cidx * 128:(cidx + 1) * 128],
                            p_l[:, cidx * 128:(cidx + 1) * 128],
                            rdg[:], start=True, stop=True)
                    pT_l = scr.tile([128, 256], BF16, tag="pTl", bufs=4)
                    nc.vector.tensor_copy(pT_l[:], ps_tpl[:])
                    ps_cl = pcl.tile([128, 128], F32, tag="cl")
                    if i:
                        nc.tensor.matmul(ps_cl[:],
                                         lvn[:, i - 1,
                                             h * 128:(h + 1) * 128],
                                         pT_l[:, 0:128],
                                         start=True, stop=False)
                        nc.tensor.matmul(ps_cl[:],
                                         lvn[:, i, h * 128:(h + 1) * 128],
                                         pT_l[:, 128:256],
                                         start=False, stop=True)
                    else:
                        nc.tensor.matmul(ps_cl[:],
                                         lvn[:, 0, h * 128:(h + 1) * 128],
                                         pT_l[:, 128:256],
                                         start=True, stop=True)
                    nc.vector.tensor_copy(
                        mgT[:, h, i * 128:(i + 1) * 128], ps_cl[:])

                pend_l = []
                for i in range(NT):
                    hT_t = stm.tile([128, KT, 128], BF16, tag="hT")
                    nc.sync.dma_start(out=hT_t[:], in_=hT.ap()[i])
                    ps_q = pq.tile([128, 512], F32)
                    ps_l = pl.tile([128, 257], F32)
                    for k in range(KT):
                        nc.tensor.matmul(ps_q[:], hT_t[:, k, :], wqlk_k(k),
                                         start=(k == 0),
                                         stop=(zero_bias and k == KT - 1))
                        nc.tensor.matmul(ps_l[:], hT_t[:, k, :], wlvc_k(k),
                                         start=(k == 0),
                                         stop=(zero_bias and k == KT - 1))
                    if not zero_bias:
                        nc.tensor.matmul(ps_q[:], ones1[:], b_qlk_sb[:],
                                         start=False, stop=True)
                        nc.tensor.matmul(ps_l[:], ones1[:], b_lvc_sb[:],
                                         start=False, stop=True)

                    # rms norm (over d) for the 4 sub-tensors [q0|q1|k0|k1]
                    ssq = sts.tile([128, 4], F32)
                    for j in range(4):
                        sq_s = scr.tile([128, 128], F32, tag="sq")
                        nc.scalar.activation(
                            sq_s[:], ps_q[:, j * 128:(j + 1) * 128],
                            mybir.ActivationFunctionType.Square,
                            accum_out=ssq[:, j:j + 1])
                    rms = sts.tile([128, 4], F32)
                    nc.scalar.activation(rms[:], ssq[:],
                                         mybir.ActivationFunctionType.Sqrt,
                                         scale=1.0 / HD, bias=eps_t[:])
                    rinv = sts.tile([128, 4], F32)
                    nc.vector.reciprocal(rinv[:], rms[:])

                    qlkn = scr.tile([128, 512], BF16, tag="qlkn", bufs=3)
                    qn = qlkn[:]
                    nc.vector.tensor_mul(
                        qn.rearrange("p (a b) -> p a b", a=4),
                        ps_q[:].rearrange("p (a b) -> p a b", a=4),
                        rinv[:].unsqueeze(2).broadcast_to([128, 4, 128]))
                    # partial rope on cols [0:64) of each sub-tensor;
                    # 4-D views: [p, sect(q/k), head, cols]
                    qn4 = qn.rearrange("p (s r b) -> p s r b", s=2, r=2)
                    x1 = qn4[:, :, :, 0:HALF]
                    x2 = qn4[:, :, :, HALF:ROPE]
                    xp = qn4[:, :, :, ROPE:128]

                    def tslice(t):
                        return (t[:, i, :]
                                .rearrange("p (s c) -> p s c", s=2)
                                .unsqueeze(2)
                                .broadcast_to([128, 2, 2, HALF]))

                    t1 = scr.tile([128, 4, HALF], BF16, tag="t1")
                    t2 = scr.tile([128, 4, HALF], BF16, tag="t2")
                    t3 = scr.tile([128, 4, HALF], BF16, tag="t3")
                    t4 = scr.tile([128, 4, HALF], BF16, tag="t4")

                    def v4(t):
                        return t[:].rearrange("p (s r) c -> p s r c", s=2)

                    nc.gpsimd.tensor_mul(v4(t1), x1, tslice(tA_sb))
                    nc.gpsimd.tensor_mul(v4(t2), x2, tslice(tB_sb))
                    nc.gpsimd.tensor_mul(v4(t3), x2, tslice(tC_sb))
                    nc.gpsimd.tensor_mul(v4(t4), x2, tslice(tD_sb))
                    nc.vector.tensor_sub(x1, v4(t1), v4(t2))
                    nc.vector.tensor_add(x2, v4(t3), v4(t4))
                    # passthrough cols [64:128) *= norm weight tail
                    nc.vector.tensor_mul(
                        xp,
                        xp,
                        qk_pass_sb.rearrange("p (s c) -> p s c", s=2)
                        .unsqueeze(2).broadcast_to([128, 2, 2, ROPE]))

                    # collect compressor scores + local V
                    nc.scalar.copy(cwN[:, i:i + 1], ps_l[:, 256:257])
                    nc.vector.tensor_copy(lvn[:, i, :], ps_l[:, 0:256])

                    # transpose q0|q1|lk0|lk1 -> qkT slot i+1 (one batch)
                    ps_t4 = ptp.tile([128, 512], BF16, tag="tp4")
                    for j in range(4):
                        nc.tensor.transpose(
                            ps_t4[:, j * 128:(j + 1) * 128],
                            qn[:, j * 128:(j + 1) * 128], ident_bf[:])
                    nc.vector.tensor_copy(qkT[:, i + 1, :], ps_t4[:])

                    # local-attention branch, software-pipelined into P1
                    for h in range(HPC):
                        cur = (i, h, *front_l(i, h))
                        if pend_l:
                            back_l(*pend_l.pop(0))
                        pend_l.append(cur)
                while pend_l:
                    back_l(*pend_l.pop(0))

                # block-softmax of compressor scores (R=16 blocks)
                ps_cw1 = ptp.tile([16, 128], F32, tag="tp4")
                nc.tensor.transpose(ps_cw1[:], cwN[:], ident_f32[:])
                cwT = scr.tile([16, 128], F32, tag="cwT")
                nc.scalar.copy(cwT[:], ps_cw1[:])
                cwE = scr.tile([16, 128], F32, tag="cwE")
                nc.scalar.activation(cwE[:], cwT[:],
                                     mybir.ActivationFunctionType.Exp)
                csum = sts.tile([16, 8], F32)
                nc.vector.tensor_reduce(
                    csum[:], cwE[:].rearrange("p (g r) -> p g r", g=8),
                    mybir.AxisListType.X, mybir.AluOpType.add)
                crec = sts.tile([16, 8], F32)
                nc.vector.reciprocal(crec[:], csum[:])
                cwW = scr.tile([16, 128], F32, tag="cwW")
                nc.vector.tensor_mul(
                    cwW[:].rearrange("p (g r) -> p g r", g=8),
                    cwE[:].rearrange("p (g r) -> p g r", g=8),
                    crec[:].unsqueeze(2).broadcast_to([16, 8, 16]))
                ps_cw2 = ptp.tile([128, 16], F32, tag="tp4")
                nc.tensor.transpose(ps_cw2[:], cwW[:], ident_f32[0:16, 0:16])
                nc.scalar.copy(wN[:], ps_cw2[:])

            # big const tail + hN pairs (used by P2 on)
            nc.sync.dma_start(out=blob[:, _BIG[0]:_BIG[1]],
                              in_=cblob.ap()[:, _BIG[0]:_BIG[1]])
            hN_tiles = []
            for j in range(NT // 2):
                hN_t = stm.tile([128, 2, HID], BF16, tag="hN", bufs=4)
                nc.sync.dma_start(out=hN_t[:], in_=hN.ap()[j])
                hN_tiles.append(hN_t)

            # ================= P2: entries + ck/cv =================
            with (
                tc.tile_pool(name="ps_e", bufs=1, space="PSUM") as pe,
                tc.tile_pool(name="ps_kv", bufs=1, space="PSUM") as pkv,
                tc.tile_pool(name="ps_tp2", bufs=2, space="PSUM") as ptp2,
            ):
                ps_e = pe.tile([C, HID], F32)
                for j in range(NT // 2):
                    hN_t = hN_tiles[j]
                    for jj in range(2):
                        i = 2 * j + jj
                        wbig = scr.tile([128, 128], BF16, tag="wbig")
                        nc.vector.tensor_scalar_mul(
                            wbig[:], dwide_sb[:, 120 - 8 * i:248 - 8 * i],
                            wN[:, i:i + 1])
                        for hc in range(4):
                            nc.tensor.matmul(
                                ps_e[:, hc * 512:(hc + 1) * 512],
                                wbig[:], hN_t[:, jj, hc * 512:(hc + 1) * 512],
                                start=(i == 0), stop=(i == NT - 1))
                for hc in range(4):
                    nc.vector.tensor_copy(entries[:, hc * 512:(hc + 1) * 512],
                                          ps_e[:, hc * 512:(hc + 1) * 512])
                for kg in range(KT // 4):
                    ps_t = ptp2.tile([128, 4, 128], BF16, tag="tp_e")
                    for kk in range(4):
                        k = kg * 4 + kk
                        nc.tensor.transpose(ps_t[:, kk, :],
                                            entries[:, k * 128:(k + 1) * 128],
                                            ident_bf[:])
                    nc.vector.tensor_copy(eT[:, kg * 4:(kg + 1) * 4, :],
                                          ps_t[:])

                ps_kv = pkv.tile([C, 256], F32)
                for k in range(KT):
                    nc.tensor.matmul(ps_kv[:], eT[:, k, :], wkv_sb[:, k, :],
                                     start=(k == 0),
                                     stop=(zero_bias and k == KT - 1))
                if not zero_bias:
                    nc.tensor.matmul(ps_kv[:], ones1[:], b_kv_sb[:],
                                     start=False, stop=True)

                # ck: rmsnorm + rope at block-end positions
                ssqc = sts.tile([C, 1], F32)
                sq_c = scr.tile([C, 128], F32, tag="sq")
                nc.scalar.activation(sq_c[:], ps_kv[:, 0:128],
                                     mybir.ActivationFunctionType.Square,
                                     accum_out=ssqc[:])
                rmsc = sts.tile([C, 1], F32)
                nc.scalar.activation(rmsc[:], ssqc[:],
                                     mybir.ActivationFunctionType.Sqrt,
                                     scale=1.0 / HD, bias=eps_t[:])
                rinvc = sts.tile([C, 1], F32)
                nc.vector.reciprocal(rinvc[:], rmsc[:])
                ckn = scr.tile([C, 128], F32, tag="ckn")
                nc.vector.tensor_scalar_mul(ckn[:], ps_kv[:, 0:128], rinvc[:])
                ckR = scr.tile([C, 128], BF16, tag="ckR")
                ct1 = scr.tile([C, HALF], F32, tag="ct1")
                ct2 = scr.tile([C, HALF], F32, tag="ct2")
                nc.vector.tensor_mul(ct1[:], ckn[:, 0:HALF], ctA_sb)
                nc.vector.tensor_mul(ct2[:], ckn[:, HALF:ROPE], ctB_sb)
                nc.vector.tensor_sub(ckR[:, 0:HALF], ct1[:], ct2[:])
                nc.vector.tensor_mul(ct1[:], ckn[:, 0:HALF], ctC_sb)
                nc.vector.tensor_mul(ct2[:], ckn[:, HALF:ROPE], ctD_sb)
                nc.vector.tensor_add(ckR[:, HALF:ROPE], ct1[:], ct2[:])
                nc.vector.tensor_mul(ckR[:, ROPE:128], ckn[:, ROPE:128],
                                     ck_pass_sb)
                nc.vector.tensor_copy(cvn[:], ps_kv[:, 128:256])
                ps_ct = ptp2.tile([128, 128], BF16, tag="tp_e")
                nc.tensor.transpose(ps_ct[:], ckR[:], ident_bf[:])
                for h in range(HPC):
                    nc.vector.tensor_copy(ckT_aug[:, h, 0:128], ps_ct[:])

            # ======== P3c+P4: compressed branch + out projection ========
            with (
                tc.tile_pool(name="ps_sc", bufs=3, space="PSUM") as psc,
                tc.tile_pool(name="ps_tpc", bufs=2, space="PSUM") as ptpc,
                tc.tile_pool(name="ps_cc", bufs=2, space="PSUM") as pcc,
                tc.tile_pool(name="ps_out", bufs=1, space="PSUM") as pout,
            ):
                def front_c(i, h):
                    qT = qkT[:, i + 1, h * 128:(h + 1) * 128]
                    ps_s = psc.tile([128, 129], F32, tag="sc")
                    nc.tensor.matmul(ps_s[:], ident_bf[:],
                                     mcmp_sb[:, i, :],
                                     start=True, stop=False)
                    nc.tensor.matmul(ps_s[:], qT, ckT_aug[:, h, :],
                                     start=False, stop=True)
                    p_c = scr.tile([128, 129], BF16, tag="pc", bufs=4)
                    den_c = sts.tile([128, 1], F32)
                    nc.scalar.activation(p_c[:], ps_s[:],
                                         mybir.ActivationFunctionType.Exp,
                                         scale=SCALE, accum_out=den_c[:])
                    return p_c, den_c

                def back_c(i, h, p_c, den_c):
                    rden = sts.tile([128, 1], F32)
                    nc.vector.reciprocal(rden[:], den_c[:])
                    rdg = scr.tile([128, 128], BF16, tag="rdgc", bufs=3)
                    nc.vector.tensor_scalar_mul(rdg[:], ident_bf[:],
                                                rden[:])
                    ps_tpc = ptpc.tile([128, 256], F32, tag="tpc")
                    nc.tensor.matmul(ps_tpc[:, 0:128], p_c[:, 0:128],
                                     rdg[:], start=True, stop=True)
                    nc.tensor.matmul(ps_tpc[0:1, 128:256], p_c[:, 128:129],
                                     rdg[:], start=True, stop=True)
                    pT_c = scr.tile([128, 256], BF16, tag="pTc", bufs=4)
                    if h == 0:
                        nc.vector.tensor_copy(pT_c[:], ps_tpc[:])
                    else:
                        nc.scalar.copy(pT_c[:], ps_tpc[:])
                    ps_cc = pcc.tile([128, 128], F32, tag="cc")
                    nc.tensor.matmul(ps_cc[:], cvn[:], pT_c[:, 0:128],
                                     start=True, stop=False)
                    nc.tensor.matmul(
                        ps_cc[:],
                        sinkv_sb[:, h * 128:(h + 1) * 128],
                        pT_c[0:1, 128:256], start=False, stop=True)
                    # merge into mgT (local ctx already there)
                    mslice = mgT[:, h, i * 128:(i + 1) * 128]
                    nc.vector.tensor_add(mslice, mslice, ps_cc[:])

                def outproj(i):
                    # out projection for s-tile i (0.5 folded into wo)
                    o_sb = scr.tile([128, HID], BF16, tag="o_sb", bufs=2)
                    for q in range(4):
                        ps_o = pout.tile([128, 512], F32, tag="ps_o")
                        for h in range(HPC):
                            nc.tensor.matmul(
                                ps_o[:],
                                mgT[:, h, i * 128:(i + 1) * 128],
                                wo_sb[:, h, q * 512:(q + 1) * 512],
                                start=(h == 0), stop=(h == HPC - 1))
                        dst = o_sb[:, q * 512:(q + 1) * 512]
                        if q % 2 == 0:
                            nc.vector.tensor_copy(dst, ps_o[:])
                        else:
                            nc.scalar.copy(dst, ps_o[:])
                    nc.sync.dma_start(out=out_p.ap()[i], in_=o_sb[:])

                # software pipeline: front(n) ... back(n-1) ... outproj(i)
                pend = None
                for i in range(NT):
                    for h in range(HPC):
                        cur = (i, h, *front_c(i, h))
                        if pend is not None:
                            back_c(*pend)
                            if pend[1] == HPC - 1:
                                outproj(pend[0])
                        pend = cur
                back_c(*pend)
                outproj(pend[0])

    nc.compile()
    return nc


# revision 28
# speedup vs baseline: 1.0499x; 1.0499x over previous
"""Trainium2 Bass kernel for HeavilyCompressedAttention.

Sharding: 16 heads across 8 cores (2 heads/core, tensor-parallel);
compressed-KV path (single shared head) replicated on every core;
out_proj row-parallel with host-side partial sum (bf16 partials).

v2: weight DMAs split per k-group for early P1 start, hN prefetched,
qkT/pT transposes batched through one PSUM tile with single wide
copies, combined per-i mask table, sink handled as rank-1 matmul,
SBUF-only elementwise ops on GpSimd, P4 fused into the P3 loop,
zero-bias fast path.
"""

import os
import sys

import numpy as np
import ml_dtypes

for _p in ("/opt/trn_rl_repo", "/root/.axon_site/_ro/trn_rl_repo"):
    if os.path.isdir(_p) and _p not in sys.path:
        sys.path.insert(0, _p)

from concourse import bacc, mybir  # noqa: E402
import concourse.tile as tile  # noqa: E402
from concourse.bass_utils import run_bass_kernel_spmd  # noqa: E402
from concourse.masks import make_identity  # noqa: E402

F32 = mybir.dt.float32
BF16 = mybir.dt.bfloat16
NPBF = ml_dtypes.bfloat16

S = 2048
HID = 2048
NH = 16
HD = 128
R = 16
C = S // R  # 128
WIN = 128
ROPE = HD // 2  # 64
HALF = ROPE // 2  # 32
EPS = 1e-6
NT = S // 128  # 16 s-tiles
KT = HID // 128  # 16 k-tiles
NCORES = 8
HPC = NH // NCORES  # 2 heads per core
SCALE = 1.0 / float(np.sqrt(HD))
MASKV = -30000.0
KG = 4  # k-tiles per weight-chunk DMA

_CACHE = {}

# ---- const blob column layout (bf16 [128, CB]) ----
_OFF = {}
_cb = 0


def _col(name, n):
    global _cb
    _OFF[name] = (_cb, _cb + n)
    _cb += n


_col("wg0", KG * 769)       # per k: [wqlk_k (512) | wlvc_k (257)]
_col("tA", NT * 64)         # --- SMALL chunk (needed early in P1) ---
_col("tB", NT * 64)
_col("tC", NT * 64)
_col("tD", NT * 64)
_col("qk_pass", 128)
_col("ctA", HALF)           # rows 0:C
_col("ctB", HALF)
_col("ctC", HALF)
_col("ctD", HALF)
_col("ck_pass", ROPE)       # rows 0:C
_col("dwide", 248)
_col("b_qlk", 512)          # row 0
_col("b_lvc", 257)          # row 0
_col("b_kv", 256)           # row 0
_col("sinkkT", HPC)         # [128, HPC] column per head
_col("sinkv", HPC * 128)    # row 0 only used (rank-1 matmul)
for _g in range(1, KT // KG):
    _col(f"wg{_g}", KG * 769)
_col("mloc", NT * 256)      # per-i local mask (used during P1)
_col("wkv", KT * 256)       # --- BIG tail (needed from P2 on) ---
_col("wo", HPC * HID)       # [p, h, HID]  (0.5 folded in)
_col("mcmp", NT * 129)      # per-i [comp 128 | sink 0] mask
CB = _cb
_SMALL = (_OFF["tA"][0], _OFF["sinkv"][1])
_BIG = (_OFF["wkv"][0], CB)


def _build_bass(zero_bias):
    nc = bacc.Bacc("TRN2", target_bir_lowering=False, debug=False,
                   num_devices=NCORES)

    din = {}

    def inp(name, shape, dt):
        din[name] = nc.dram_tensor(name, list(shape), dt, kind="ExternalInput")
        return din[name]

    hT = inp("hT", [NT, 128, KT * 128], BF16)   # [i][hid_p][k*128+c(s)]
    hN = inp("hN", [NT // 2, 128, 2 * HID], BF16)  # [j][s_p][jj*HID+hid]
    cblob = inp("cblob", [128, CB], BF16)

    out_p = nc.dram_tensor("out_p", [NT, 128, HID], BF16,
                           kind="ExternalOutput")

    with tile.TileContext(nc) as tc:
        with (
            tc.tile_pool(name="const", bufs=1) as cst,
            tc.tile_pool(name="persist", bufs=1) as per,
            tc.tile_pool(name="stream", bufs=3) as stm,
            tc.tile_pool(name="scratch", bufs=3) as scr,
            tc.tile_pool(name="stats", bufs=6) as sts,
        ):
            blob = cst.tile([128, CB], BF16, name="c_blob")
            # wg0 + small tables first (P1 start), wg1..3 next;
            # big tail (wkv|wo|mcomb) emitted just before P2
            a, b = _OFF["wg0"]
            nc.sync.dma_start(out=blob[:, a:b], in_=cblob.ap()[:, a:b])
            nc.sync.dma_start(out=blob[:, _SMALL[0]:_SMALL[1]],
                              in_=cblob.ap()[:, _SMALL[0]:_SMALL[1]])
            for g in range(1, KT // KG):
                a, b = _OFF[f"wg{g}"]
                nc.sync.dma_start(out=blob[:, a:b],
                                  in_=cblob.ap()[:, a:b])
            a, b = _OFF["mloc"]
            nc.sync.dma_start(out=blob[:, a:b], in_=cblob.ap()[:, a:b])

            def cv(name):
                a, b = _OFF[name]
                return blob[:, a:b]

            # weight views: wqlk k -> wg{k//KG} cols [(k%KG)*769, +512]
            def wqlk_k(k):
                base = _OFF[f"wg{k // KG}"][0] + (k % KG) * 769
                return blob[:, base:base + 512]

            def wlvc_k(k):
                base = _OFF[f"wg{k // KG}"][0] + (k % KG) * 769 + 512
                return blob[:, base:base + 257]

            wkv_sb = cv("wkv").rearrange("p (k n) -> p k n", k=KT)
            wo_sb = cv("wo").rearrange("p (h n) -> p h n", h=HPC)
            tA_sb = cv("tA").rearrange("p (i n) -> p i n", i=NT)
            tB_sb = cv("tB").rearrange("p (i n) -> p i n", i=NT)
            tC_sb = cv("tC").rearrange("p (i n) -> p i n", i=NT)
            tD_sb = cv("tD").rearrange("p (i n) -> p i n", i=NT)
            qk_pass_sb = cv("qk_pass")
            ctA_sb = cv("ctA")[0:C, :]
            ctB_sb = cv("ctB")[0:C, :]
            ctC_sb = cv("ctC")[0:C, :]
            ctD_sb = cv("ctD")[0:C, :]
            ck_pass_sb = cv("ck_pass")[0:C, :]
            mloc_sb = cv("mloc").rearrange("p (i n) -> p i n", i=NT)
            mcmp_sb = cv("mcmp").rearrange("p (i n) -> p i n", i=NT)
            dwide_sb = cv("dwide")
            b_qlk_sb = cv("b_qlk")[0:1, :]
            b_lvc_sb = cv("b_lvc")[0:1, :]
            b_kv_sb = cv("b_kv")[0:1, :]
            sinkkT_sb = cv("sinkkT")
            sinkv_sb = cv("sinkv")[0:1, :]

            ident_bf = cst.tile([128, 128], BF16)
            make_identity(nc, ident_bf[:])
            ident_f32 = cst.tile([128, 128], F32)
            make_identity(nc, ident_f32[:])
            ones1 = cst.tile([1, 128], BF16)
            nc.vector.memset(ones1[:], 1.0)
            eps_t = cst.tile([128, 1], F32)
            nc.vector.memset(eps_t[:], EPS)

            # ---- persistent activations ----
            lvn = per.tile([128, NT, 256], BF16)    # local v (natural)
            # q0|q1|lk0|lk1 transposed, slot i+1 = tile i; slot0 lk = 0
            qkT = per.tile([128, NT + 1, 512], BF16)
            cwN = per.tile([128, NT], F32)          # compressor scores
            wN = per.tile([128, NT], F32)           # block-softmaxed weights
            entries = per.tile([C, HID], BF16)
            eT = per.tile([128, KT, C], BF16)       # entries^T tiles
            cvn = per.tile([C, 128], BF16)
            ckT_aug = per.tile([128, HPC, 129], BF16)  # ck^T | sink_k col
            mgT = per.tile([128, HPC, S], BF16)     # merged^T = cc+cl

            nc.vector.memset(qkT[:, 0, 256:512], 0.0)
            for h in range(HPC):
                nc.vector.tensor_copy(ckT_aug[:, h, 128:129],
                                      sinkkT_sb[:, h:h + 1])

            # ================= P1: projections + norm/rope =================
            with (
                tc.tile_pool(name="ps_qlk", bufs=2, space="PSUM") as pq,
                tc.tile_pool(name="ps_lvc", bufs=2, space="PSUM") as pl,
                tc.tile_pool(name="ps_tp", bufs=1, space="PSUM") as ptp,
                tc.tile_pool(name="ps_sl", bufs=1, space="PSUM") as psl,
                tc.tile_pool(name="ps_tpl", bufs=1, space="PSUM") as ptpl,
                tc.tile_pool(name="ps_cl", bufs=1, space="PSUM") as pcl,
            ):
                def front_l(i, h):
                    # local-branch scores+mask+softmax for pair (i, h)
                    qT = qkT[:, i + 1, h * 128:(h + 1) * 128]
                    ps_sl = psl.tile([128, 256], F32, tag="sl")
                    nc.tensor.matmul(ps_sl[:], ident_bf[:],
                                     mloc_sb[:, i, :],
                                     start=True, stop=False)
                    nc.tensor.matmul(
                        ps_sl[:], qT,
                        qkT[:, i:i + 2, 256 + h * 128:256 + (h + 1) * 128],
                        start=False, stop=True)
                    p_l = scr.tile([128, 256], BF16, tag="pl", bufs=4)
                    den_l = sts.tile([128, 1], F32)
                    nc.scalar.activation(p_l[:], ps_sl[:],
                                         mybir.ActivationFunctionType.Exp,
                                         scale=SCALE, accum_out=den_l[:])
                    return p_l, den_l

                def back_l(i, h, p_l, den_l):
                    rden = sts.tile([128, 1], F32)
                    nc.vector.reciprocal(rden[:], den_l[:])
                    rdg = scr.tile([128, 128], BF16, tag="rdgl", bufs=3)
                    nc.vector.tensor_scalar_mul(rdg[:], ident_bf[:],
                                                rden[:])
                    ps_tpl = ptpl.tile([128, 256], F32, tag="tpl")
                    for cidx in range(2):
                        nc.tensor.matmul(
                            ps_tpl[:, cidx * 128:(cidx + 1) * 128],
                            p_l[:, cidx * 128:(cidx + 1) * 128],
                            rdg[:], start=True, stop=True)
                    pT_l = scr.tile([128, 256], BF16, tag="pTl", bufs=4)
                    nc.vector.tensor_copy(pT_l[:], ps_tpl[:])
                    ps_cl = pcl.tile([128, 128], F32, tag="cl")
                    if i:
                        nc.tensor.matmul(ps_cl[:],
                                         lvn[:, i - 1,
                                             h * 128:(h + 1) * 128],
                                         pT_l[:, 0:128],
                                         start=True, stop=False)
                        nc.tensor.matmul(ps_cl[:],
                                         lvn[:, i, h * 128:(h + 1) * 128],
                                         pT_l[:, 128:256],
                                         start=False, stop=True)
                    else:
                        nc.tensor.matmul(ps_cl[:],
                                         lvn[:, 0, h * 128:(h + 1) * 128],
                                         pT_l[:, 128:256],
                                         start=True, stop=True)
                    nc.vector.tensor_copy(
                        mgT[:, h, i * 128:(i + 1) * 128], ps_cl[:])

                pend_l = []
                for i in range(NT):
                    hT_t = stm.tile([128, KT, 128], BF16, tag="hT")
                    nc.sync.dma_start(out=hT_t[:], in_=hT.ap()[i])
                    ps_q = pq.tile([128, 512], F32)
                    ps_l = pl.tile([128, 257], F32)
                    for k in range(KT):
                        nc.tensor.matmul(ps_q[:], hT_t[:, k, :], wqlk_k(k),
                                         start=(k == 0),
                                         stop=(zero_bias and k == KT - 1))
                        nc.tensor.matmul(ps_l[:], hT_t[:, k, :], wlvc_k(k),
                                         start=(k == 0),
                                         stop=(zero_bias and k == KT - 1))
                    if not zero_bias:
                        nc.tensor.matmul(ps_q[:], ones1[:], b_qlk_sb[:],
                                         start=False, stop=True)
                        nc.tensor.matmul(ps_l[:], ones1[:], b_lvc_sb[:],
                                         start=False, stop=True)

                    # rms norm (over d) for the 4 sub-tensors [q0|q1|k0|k1]
                    ssq = sts.tile([128, 4], F32)
                    for j in range(4):
                        sq_s = scr.tile([128, 128], F32, tag="sq")
                        nc.scalar.activation(
                            sq_s[:], ps_q[:, j * 128:(j + 1) * 128],
                            mybir.ActivationFunctionType.Square,
                            accum_out=ssq[:, j:j + 1])
                    rms = sts.tile([128, 4], F32)
                    nc.scalar.activation(rms[:], ssq[:],
                                         mybir.ActivationFunctionType.Sqrt,
                                         scale=1.0 / HD, bias=eps_t[:])
                    rinv = sts.tile([128, 4], F32)
                    nc.vector.reciprocal(rinv[:], rms[:])

                    qlkn = scr.tile([128, 512], BF16, tag="qlkn", bufs=3)
                    qn = qlkn[:]
                    nc.vector.tensor_mul(
                        qn.rearrange("p (a b) -> p a b", a=4),
                        ps_q[:].rearrange("p (a b) -> p a b", a=4),
                        rinv[:].unsqueeze(2).broadcast_to([128, 4, 128]))
                    # partial rope on cols [0:64) of each sub-tensor;
                    # 4-D views: [p, sect(q/k), head, cols]
                    qn4 = qn.rearrange("p (s r b) -> p s r b", s=2, r=2)
                    x1 = qn4[:, :, :, 0:HALF]
                    x2 = qn4[:, :, :, HALF:ROPE]
                    xp = qn4[:, :, :, ROPE:128]

                    def tslice(t):
                        return (t[:, i, :]
                                .rearrange("p (s c) -> p s c", s=2)
                                .unsqueeze(2)
                                .broadcast_to([128, 2, 2, HALF]))

                    t1 = scr.tile([128, 4, HALF], BF16, tag="t1")
                    t2 = scr.tile([128, 4, HALF], BF16, tag="t2")
                    t3 = scr.tile([128, 4, HALF], BF16, tag="t3")
                    t4 = scr.tile([128, 4, HALF], BF16, tag="t4")

                    def v4(t):
                        return t[:].rearrange("p (s r) c -> p s r c", s=2)

                    nc.vector.tensor_mul(v4(t1), x1, tslice(tA_sb))
                    nc.vector.tensor_mul(v4(t2), x2, tslice(tB_sb))
                    nc.vector.tensor_mul(v4(t3), x1, tslice(tC_sb))
                    nc.vector.tensor_mul(v4(t4), x2, tslice(tD_sb))
                    nc.vector.tensor_sub(x1, v4(t1), v4(t2))
                    nc.vector.tensor_add(x2, v4(t3), v4(t4))
                    # passthrough cols [64:128) *= norm weight tail
                    nc.vector.tensor_mul(
                        xp,
                        xp,
                        qk_pass_sb.rearrange("p (s c) -> p s c", s=2)
                        .unsqueeze(2).broadcast_to([128, 2, 2, ROPE]))

                    # collect compressor scores + local V
                    nc.scalar.copy(cwN[:, i:i + 1], ps_l[:, 256:257])
                    nc.vector.tensor_copy(lvn[:, i, :], ps_l[:, 0:256])

                    # transpose q0|q1|lk0|lk1 -> qkT slot i+1 (one batch)
                    ps_t4 = ptp.tile([128, 512], BF16, tag="tp4")
                    for j in range(4):
                        nc.tensor.transpose(
                            ps_t4[:, j * 128:(j + 1) * 128],
                            qn[:, j * 128:(j + 1) * 128], ident_bf[:])
                    nc.vector.tensor_copy(qkT[:, i + 1, :], ps_t4[:])

                    # local-attention branch, software-pipelined into P1
                    for h in range(HPC):
                        cur = (i, h, *front_l(i, h))
                        if pend_l:
                            back_l(*pend_l.pop(0))
                        pend_l.append(cur)
                while pend_l:
                    back_l(*pend_l.pop(0))

                # block-softmax of compressor scores (R=16 blocks)
                ps_cw1 = ptp.tile([16, 128], F32, tag="tp4")
                nc.tensor.transpose(ps_cw1[:], cwN[:], ident_f32[:])
                cwT = scr.tile([16, 128], F32, tag="cwT")
                nc.scalar.copy(cwT[:], ps_cw1[:])
                cwE = scr.tile([16, 128], F32, tag="cwE")
                nc.scalar.activation(cwE[:], cwT[:],
                                     mybir.ActivationFunctionType.Exp)
                csum = sts.tile([16, 8], F32)
                nc.vector.tensor_reduce(
                    csum[:], cwE[:].rearrange("p (g r) -> p g r", g=8),
                    mybir.AxisListType.X, mybir.AluOpType.add)
                crec = sts.tile([16, 8], F32)
                nc.vector.reciprocal(crec[:], csum[:])
                cwW = scr.tile([16, 128], F32, tag="cwW")
                nc.vector.tensor_mul(
                    cwW[:].rearrange("p (g r) -> p g r", g=8),
                    cwE[:].rearrange("p (g r) -> p g r", g=8),
                    crec[:].unsqueeze(2).broadcast_to([16, 8, 16]))
                ps_cw2 = ptp.tile([128, 16], F32, tag="tp4")
                nc.tensor.transpose(ps_cw2[:], cwW[:], ident_f32[0:16, 0:16])
                nc.scalar.copy(wN[:], ps_cw2[:])

            # big const tail + hN pairs (used by P2 on)
            nc.sync.dma_start(out=blob[:, _BIG[0]:_BIG[1]],
                              in_=cblob.ap()[:, _BIG[0]:_BIG[1]])
            hN_tiles = []
            for j in range(NT // 2):
                hN_t = stm.tile([128, 2, HID], BF16, tag="hN", bufs=4)
                nc.sync.dma_start(out=hN_t[:], in_=hN.ap()[j])
                hN_tiles.append(hN_t)

            # ================= P2: entries + ck/cv =================
            with (
                tc.tile_pool(name="ps_e", bufs=1, space="PSUM") as pe,
                tc.tile_pool(name="ps_kv", bufs=1, space="PSUM") as pkv,
                tc.tile_pool(name="ps_tp2", bufs=2, space="PSUM") as ptp2,
            ):
                ps_e = pe.tile([C, HID], F32)
                for j in range(NT // 2):
                    hN_t = hN_tiles[j]
                    for jj in range(2):
                        i = 2 * j + jj
                        wbig = scr.tile([128, 128], BF16, tag="wbig")
                        nc.vector.tensor_scalar_mul(
                            wbig[:], dwide_sb[:, 120 - 8 * i:248 - 8 * i],
                            wN[:, i:i + 1])
                        for hc in range(4):
                            nc.tensor.matmul(
                                ps_e[:, hc * 512:(hc + 1) * 512],
                                wbig[:], hN_t[:, jj, hc * 512:(hc + 1) * 512],
                                start=(i == 0), stop=(i == NT - 1))
                for hc in range(4):
                    nc.vector.tensor_copy(entries[:, hc * 512:(hc + 1) * 512],
                                          ps_e[:, hc * 512:(hc + 1) * 512])
                for kg in range(KT // 4):
                    ps_t = ptp2.tile([128, 4, 128], BF16, tag="tp_e")
                    for kk in range(4):
                        k = kg * 4 + kk
                        nc.tensor.transpose(ps_t[:, kk, :],
                                            entries[:, k * 128:(k + 1) * 128],
                                            ident_bf[:])
                    nc.vector.tensor_copy(eT[:, kg * 4:(kg + 1) * 4, :],
                                          ps_t[:])

                ps_kv = pkv.tile([C, 256], F32)
                for k in range(KT):
                    nc.tensor.matmul(ps_kv[:], eT[:, k, :], wkv_sb[:, k, :],
                                     start=(k == 0),
                                     stop=(zero_bias and k == KT - 1))
                if not zero_bias:
                    nc.tensor.matmul(ps_kv[:], ones1[:], b_kv_sb[:],
                                     start=False, stop=True)

                # ck: rmsnorm + rope at block-end positions
                ssqc = sts.tile([C, 1], F32)
                sq_c = scr.tile([C, 128], F32, tag="sq")
                nc.scalar.activation(sq_c[:], ps_kv[:, 0:128],
                                     mybir.ActivationFunctionType.Square,
                                     accum_out=ssqc[:])
                rmsc = sts.tile([C, 1], F32)
                nc.scalar.activation(rmsc[:], ssqc[:],
                                     mybir.ActivationFunctionType.Sqrt,
                                     scale=1.0 / HD, bias=eps_t[:])
                rinvc = sts.tile([C, 1], F32)
                nc.vector.reciprocal(rinvc[:], rmsc[:])
                ckn = scr.tile([C, 128], F32, tag="ckn")
                nc.vector.tensor_scalar_mul(ckn[:], ps_kv[:, 0:128], rinvc[:])
                ckR = scr.tile([C, 128], BF16, tag="ckR")
                ct1 = scr.tile([C, HALF], F32, tag="ct1")
                ct2 = scr.tile([C, HALF], F32, tag="ct2")
                nc.vector.tensor_mul(ct1[:], ckn[:, 0:HALF], ctA_sb)
                nc.vector.tensor_mul(ct2[:], ckn[:, HALF:ROPE], ctB_sb)
                nc.vector.tensor_sub(ckR[:, 0:HALF], ct1[:], ct2[:])
                nc.vector.tensor_mul(ct1[:], ckn[:, 0:HALF], ctC_sb)
                nc.vector.tensor_mul(ct2[:], ckn[:, HALF:ROPE], ctD_sb)
                nc.vector.tensor_add(ckR[:, HALF:ROPE], ct1[:], ct2[:])
                nc.vector.tensor_mul(ckR[:, ROPE:128], ckn[:, ROPE:128],
                                     ck_pass_sb)
                nc.vector.tensor_copy(cvn[:], ps_kv[:, 128:256])
                ps_ct = ptp2.tile([128, 128], BF16, tag="tp_e")
                nc.tensor.transpose(ps_ct[:], ckR[:], ident_bf[:])
                for h in range(HPC):
                    nc.vector.tensor_copy(ckT_aug[:, h, 0:128], ps_ct[:])

            # ======== P3c+P4: compressed branch + out projection ========
            with (
                tc.tile_pool(name="ps_sc", bufs=3, space="PSUM") as psc,
                tc.tile_pool(name="ps_tpc", bufs=2, space="PSUM") as ptpc,
                tc.tile_pool(name="ps_cc", bufs=2, space="PSUM") as pcc,
                tc.tile_pool(name="ps_out", bufs=1, space="PSUM") as pout,
            ):
                def front_c(i, h):
                    qT = qkT[:, i + 1, h * 128:(h + 1) * 128]
                    ps_s = psc.tile([128, 129], F32, tag="sc")
                    nc.tensor.matmul(ps_s[:], ident_bf[:],
                                     mcmp_sb[:, i, :],
                                     start=True, stop=False)
                    nc.tensor.matmul(ps_s[:], qT, ckT_aug[:, h, :],
                                     start=False, stop=True)
                    p_c = scr.tile([128, 129], BF16, tag="pc", bufs=4)
                    den_c = sts.tile([128, 1], F32)
                    nc.scalar.activation(p_c[:], ps_s[:],
                                         mybir.ActivationFunctionType.Exp,
                                         scale=SCALE, accum_out=den_c[:])
                    return p_c, den_c

                def back_c(i, h, p_c, den_c):
                    rden = sts.tile([128, 1], F32)
                    nc.vector.reciprocal(rden[:], den_c[:])
                    rdg = scr.tile([128, 128], BF16, tag="rdgc", bufs=3)
                    nc.vector.tensor_scalar_mul(rdg[:], ident_bf[:],
                                                rden[:])
                    ps_tpc = ptpc.tile([128, 256], F32, tag="tpc")
                    nc.tensor.matmul(ps_tpc[:, 0:128], p_c[:, 0:128],
                                     rdg[:], start=True, stop=True)
                    nc.tensor.matmul(ps_tpc[0:1, 128:256], p_c[:, 128:129],
                                     rdg[:], start=True, stop=True)
                    pT_c = scr.tile([128, 256], BF16, tag="pTc", bufs=4)
                    if h == 0:
                        nc.vector.tensor_copy(pT_c[:], ps_tpc[:])
                    else:
                        nc.scalar.copy(pT_c[:], ps_tpc[:])
                    ps_cc = pcc.tile([128, 128], F32, tag="cc")
                    nc.tensor.matmul(ps_cc[:], cvn[:], pT_c[:, 0:128],
                                     start=True, stop=False)
                    nc.tensor.matmul(
                        ps_cc[:],
                        sinkv_sb[:, h * 128:(h + 1) * 128],
                        pT_c[0:1, 128:256], start=False, stop=True)
                    # merge into mgT (local ctx already there)
                    mslice = mgT[:, h, i * 128:(i + 1) * 128]
                    nc.vector.tensor_add(mslice, mslice, ps_cc[:])

                def outproj(i):
                    # out projection for s-tile i (0.5 folded into wo)
                    o_sb = scr.tile([128, HID], BF16, tag="o_sb", bufs=2)
                    for q in range(4):
                        ps_o = pout.tile([128, 512], F32, tag="ps_o")
                        for h in range(HPC):
                            nc.tensor.matmul(
                                ps_o[:],
                                mgT[:, h, i * 128:(i + 1) * 128],
                                wo_sb[:, h, q * 512:(q + 1) * 512],
                                start=(h == 0), stop=(h == HPC - 1))
                        dst = o_sb[:, q * 512:(q + 1) * 512]
                        if q % 2 == 0:
                            nc.vector.tensor_copy(dst, ps_o[:])
                        else:
                            nc.scalar.copy(dst, ps_o[:])
                    nc.sync.dma_start(out=out_p.ap()[i], in_=o_sb[:])

                # software pipeline: front(n) ... back(n-1) ... outproj(i)
                pend = None
                for i in range(NT):
                    for h in range(HPC):
                        cur = (i, h, *front_c(i, h))
                        if pend is not None:
                            back_c(*pend)
                            if pend[1] == HPC - 1:
                                outproj(pend[0])
                        pend = cur
                back_c(*pend)
                outproj(pend[0])

    nc.compile()
    return nc


# revision 29
# speedup vs baseline: 1.0549x; 1.0047x over previous
"""Trainium2 Bass kernel for HeavilyCompressedAttention.

Sharding: 16 heads across 8 cores (2 heads/core, tensor-parallel);
compressed-KV path (single shared head) replicated on every core;
out_proj row-parallel with host-side partial sum (bf16 partials).

v2: weight DMAs split per k-group for early P1 start, hN prefetched,
qkT/pT transposes batched through one PSUM tile with single wide
copies, combined per-i mask table, sink handled as rank-1 matmul,
SBUF-only elementwise ops on GpSimd, P4 fused into the P3 loop,
zero-bias fast path.
"""

import os
import sys

import numpy as np
import ml_dtypes

for _p in ("/opt/trn_rl_repo", "/root/.axon_site/_ro/trn_rl_repo"):
    if os.path.isdir(_p) and _p not in sys.path:
        sys.path.insert(0, _p)

from concourse import bacc, mybir  # noqa: E402
import concourse.tile as tile  # noqa: E402
from concourse.bass_utils import run_bass_kernel_spmd  # noqa: E402
from concourse.masks import make_identity  # noqa: E402

F32 = mybir.dt.float32
BF16 = mybir.dt.bfloat16
NPBF = ml_dtypes.bfloat16

S = 2048
HID = 2048
NH = 16
HD = 128
R = 16
C = S // R  # 128
WIN = 128
ROPE = HD // 2  # 64
HALF = ROPE // 2  # 32
EPS = 1e-6
NT = S // 128  # 16 s-tiles
KT = HID // 128  # 16 k-tiles
NCORES = 8
HPC = NH // NCORES  # 2 heads per core
SCALE = 1.0 / float(np.sqrt(HD))
MASKV = -30000.0
KG = 4  # k-tiles per weight-chunk DMA

_CACHE = {}

# ---- const blob column layout (bf16 [128, CB]) ----
_OFF = {}
_cb = 0


def _col(name, n):
    global _cb
    _OFF[name] = (_cb, _cb + n)
    _cb += n


_col("wg0", KG * 769)       # per k: [wqlk_k (512) | wlvc_k (257)]
_col("tA", NT * 64)         # --- SMALL chunk (needed early in P1) ---
_col("tB", NT * 64)
_col("tC", NT * 64)
_col("tD", NT * 64)
_col("qk_pass", 128)
_col("ctA", HALF)           # rows 0:C
_col("ctB", HALF)
_col("ctC", HALF)
_col("ctD", HALF)
_col("ck_pass", ROPE)       # rows 0:C
_col("dwide", 248)
_col("b_qlk", 512)          # row 0
_col("b_lvc", 257)          # row 0
_col("b_kv", 256)           # row 0
_col("sinkkT", HPC)         # [128, HPC] column per head
_col("sinkv", HPC * 128)    # row 0 only used (rank-1 matmul)
for _g in range(1, KT // KG):
    _col(f"wg{_g}", KG * 769)
_col("mloc", NT * 256)      # per-i local mask (used during P1)
_col("wkv", KT * 256)       # --- BIG tail (needed from P2 on) ---
_col("wo", HPC * HID)       # [p, h, HID]  (0.5 folded in)
_col("mcmp", NT * 129)      # per-i [comp 128 | sink 0] mask
CB = _cb
_SMALL = (_OFF["tA"][0], _OFF["sinkv"][1])
_BIG = (_OFF["wkv"][0], CB)


def _build_bass(zero_bias):
    nc = bacc.Bacc("TRN2", target_bir_lowering=False, debug=False,
                   num_devices=NCORES)

    din = {}

    def inp(name, shape, dt):
        din[name] = nc.dram_tensor(name, list(shape), dt, kind="ExternalInput")
        return din[name]

    hT = inp("hT", [NT, 128, KT * 128], BF16)   # [i][hid_p][k*128+c(s)]
    hN = inp("hN", [NT // 2, 128, 2 * HID], BF16)  # [j][s_p][jj*HID+hid]
    cblob = inp("cblob", [128, CB], BF16)

    out_p = nc.dram_tensor("out_p", [NT, 128, HID], BF16,
                           kind="ExternalOutput")

    with tile.TileContext(nc) as tc:
        with (
            tc.tile_pool(name="const", bufs=1) as cst,
            tc.tile_pool(name="persist", bufs=1) as per,
            tc.tile_pool(name="stream", bufs=3) as stm,
            tc.tile_pool(name="scratch", bufs=3) as scr,
            tc.tile_pool(name="stats", bufs=6) as sts,
        ):
            blob = cst.tile([128, CB], BF16, name="c_blob")
            # wg0 + small tables first (P1 start), wg1..3 next;
            # big tail (wkv|wo|mcomb) emitted just before P2
            a, b = _OFF["wg0"]
            nc.sync.dma_start(out=blob[:, a:b], in_=cblob.ap()[:, a:b])
            nc.sync.dma_start(out=blob[:, _SMALL[0]:_SMALL[1]],
                              in_=cblob.ap()[:, _SMALL[0]:_SMALL[1]])
            for g in range(1, KT // KG):
                a, b = _OFF[f"wg{g}"]
                nc.sync.dma_start(out=blob[:, a:b],
                                  in_=cblob.ap()[:, a:b])
            a, b = _OFF["mloc"]
            nc.sync.dma_start(out=blob[:, a:b], in_=cblob.ap()[:, a:b])

            def cv(name):
                a, b = _OFF[name]
                return blob[:, a:b]

            # weight views: wqlk k -> wg{k//KG} cols [(k%KG)*769, +512]
            def wqlk_k(k):
                base = _OFF[f"wg{k // KG}"][0] + (k % KG) * 769
                return blob[:, base:base + 512]

            def wlvc_k(k):
                base = _OFF[f"wg{k // KG}"][0] + (k % KG) * 769 + 512
                return blob[:, base:base + 257]

            wkv_sb = cv("wkv").rearrange("p (k n) -> p k n", k=KT)
            wo_sb = cv("wo").rearrange("p (h n) -> p h n", h=HPC)
            tA_sb = cv("tA").rearrange("p (i n) -> p i n", i=NT)
            tB_sb = cv("tB").rearrange("p (i n) -> p i n", i=NT)
            tC_sb = cv("tC").rearrange("p (i n) -> p i n", i=NT)
            tD_sb = cv("tD").rearrange("p (i n) -> p i n", i=NT)
            qk_pass_sb = cv("qk_pass")
            ctA_sb = cv("ctA")[0:C, :]
            ctB_sb = cv("ctB")[0:C, :]
            ctC_sb = cv("ctC")[0:C, :]
            ctD_sb = cv("ctD")[0:C, :]
            ck_pass_sb = cv("ck_pass")[0:C, :]
            mloc_sb = cv("mloc").rearrange("p (i n) -> p i n", i=NT)
            mcmp_sb = cv("mcmp").rearrange("p (i n) -> p i n", i=NT)
            dwide_sb = cv("dwide")
            b_qlk_sb = cv("b_qlk")[0:1, :]
            b_lvc_sb = cv("b_lvc")[0:1, :]
            b_kv_sb = cv("b_kv")[0:1, :]
            sinkkT_sb = cv("sinkkT")
            sinkv_sb = cv("sinkv")[0:1, :]

            ident_bf = cst.tile([128, 128], BF16)
            make_identity(nc, ident_bf[:])
            ident_f32 = cst.tile([128, 128], F32)
            make_identity(nc, ident_f32[:])
            ones1 = cst.tile([1, 128], BF16)
            nc.vector.memset(ones1[:], 1.0)
            eps_t = cst.tile([128, 1], F32)
            nc.vector.memset(eps_t[:], EPS)

            # ---- persistent activations ----
            lvn = per.tile([128, NT, 256], BF16)    # local v (natural)
            # q0|q1|lk0|lk1 transposed, slot i+1 = tile i; slot0 lk = 0
            qkT = per.tile([128, NT + 1, 512], BF16)
            cwN = per.tile([128, NT], F32)          # compressor scores
            wN = per.tile([128, NT], F32)           # block-softmaxed weights
            entries = per.tile([C, HID], BF16)
            eT = per.tile([128, KT, C], BF16)       # entries^T tiles
            cvn = per.tile([C, 128], BF16)
            ckT_aug = per.tile([128, HPC, 129], BF16)  # ck^T | sink_k col
            mgT = per.tile([128, HPC, S], BF16)     # merged^T = cc+cl

            nc.vector.memset(qkT[:, 0, 256:512], 0.0)
            for h in range(HPC):
                nc.vector.tensor_copy(ckT_aug[:, h, 128:129],
                                      sinkkT_sb[:, h:h + 1])

            # ================= P1: projections + norm/rope =================
            with (
                tc.tile_pool(name="ps_qlk", bufs=2, space="PSUM") as pq,
                tc.tile_pool(name="ps_lvc", bufs=2, space="PSUM") as pl,
                tc.tile_pool(name="ps_tp", bufs=1, space="PSUM") as ptp,
                tc.tile_pool(name="ps_sl", bufs=1, space="PSUM") as psl,
                tc.tile_pool(name="ps_tpl", bufs=1, space="PSUM") as ptpl,
                tc.tile_pool(name="ps_cl", bufs=1, space="PSUM") as pcl,
            ):
                def front_l(i, h):
                    # local-branch scores+mask+softmax for pair (i, h)
                    qT = qkT[:, i + 1, h * 128:(h + 1) * 128]
                    ps_sl = psl.tile([128, 256], F32, tag="sl")
                    nc.tensor.matmul(ps_sl[:], ident_bf[:],
                                     mloc_sb[:, i, :],
                                     start=True, stop=False)
                    nc.tensor.matmul(
                        ps_sl[:], qT,
                        qkT[:, i:i + 2, 256 + h * 128:256 + (h + 1) * 128],
                        start=False, stop=True)
                    p_l = scr.tile([128, 256], BF16, tag="pl", bufs=4)
                    den_l = sts.tile([128, 1], F32)
                    nc.scalar.activation(p_l[:], ps_sl[:],
                                         mybir.ActivationFunctionType.Exp,
                                         scale=SCALE, accum_out=den_l[:])
                    return p_l, den_l

                def back_l(i, h, p_l, den_l):
                    rden = sts.tile([128, 1], F32)
                    nc.vector.reciprocal(rden[:], den_l[:])
                    rdg = scr.tile([128, 128], BF16, tag="rdgl", bufs=3)
                    nc.vector.tensor_scalar_mul(rdg[:], ident_bf[:],
                                                rden[:])
                    ps_tpl = ptpl.tile([128, 256], F32, tag="tpl")
                    for cidx in range(2):
                        nc.tensor.matmul(
                            ps_tpl[:, cidx * 128:(cidx + 1) * 128],
                            p_l[:, cidx * 128:(cidx + 1) * 128],
                            rdg[:], start=True, stop=True)
                    pT_l = scr.tile([128, 256], BF16, tag="pTl", bufs=4)
                    nc.vector.tensor_copy(pT_l[:], ps_tpl[:])
                    ps_cl = pcl.tile([128, 128], F32, tag="cl")
                    if i:
                        nc.tensor.matmul(ps_cl[:],
                                         lvn[:, i - 1,
                                             h * 128:(h + 1) * 128],
                                         pT_l[:, 0:128],
                                         start=True, stop=False)
                        nc.tensor.matmul(ps_cl[:],
                                         lvn[:, i, h * 128:(h + 1) * 128],
                                         pT_l[:, 128:256],
                                         start=False, stop=True)
                    else:
                        nc.tensor.matmul(ps_cl[:],
                                         lvn[:, 0, h * 128:(h + 1) * 128],
                                         pT_l[:, 128:256],
                                         start=True, stop=True)
                    nc.vector.tensor_copy(
                        mgT[:, h, i * 128:(i + 1) * 128], ps_cl[:])

                pend_l = []
                for i in range(NT):
                    hT_t = stm.tile([128, KT, 128], BF16, tag="hT")
                    nc.sync.dma_start(out=hT_t[:], in_=hT.ap()[i])
                    ps_q = pq.tile([128, 512], F32)
                    ps_l = pl.tile([128, 257], F32)
                    for k in range(KT):
                        nc.tensor.matmul(ps_q[:], hT_t[:, k, :], wqlk_k(k),
                                         start=(k == 0),
                                         stop=(zero_bias and k == KT - 1))
                        nc.tensor.matmul(ps_l[:], hT_t[:, k, :], wlvc_k(k),
                                         start=(k == 0),
                                         stop=(zero_bias and k == KT - 1))
                    if not zero_bias:
                        nc.tensor.matmul(ps_q[:], ones1[:], b_qlk_sb[:],
                                         start=False, stop=True)
                        nc.tensor.matmul(ps_l[:], ones1[:], b_lvc_sb[:],
                                         start=False, stop=True)

                    # rms norm (over d) for the 4 sub-tensors [q0|q1|k0|k1]
                    ssq = sts.tile([128, 4], F32)
                    sq_s = scr.tile([128, 512], F32, tag="sq")
                    nc.scalar.activation(sq_s[:], ps_q[:],
                                         mybir.ActivationFunctionType.Square)
                    nc.vector.tensor_reduce(
                        ssq[:], sq_s[:].rearrange("p (a b) -> p a b", a=4),
                        mybir.AxisListType.X, mybir.AluOpType.add)
                    rms = sts.tile([128, 4], F32)
                    nc.scalar.activation(rms[:], ssq[:],
                                         mybir.ActivationFunctionType.Sqrt,
                                         scale=1.0 / HD, bias=eps_t[:])
                    rinv = sts.tile([128, 4], F32)
                    nc.vector.reciprocal(rinv[:], rms[:])

                    qlkn = scr.tile([128, 512], BF16, tag="qlkn", bufs=3)
                    qn = qlkn[:]
                    nc.vector.tensor_mul(
                        qn.rearrange("p (a b) -> p a b", a=4),
                        ps_q[:].rearrange("p (a b) -> p a b", a=4),
                        rinv[:].unsqueeze(2).broadcast_to([128, 4, 128]))
                    # partial rope on cols [0:64) of each sub-tensor;
                    # 4-D views: [p, sect(q/k), head, cols]
                    qn4 = qn.rearrange("p (s r b) -> p s r b", s=2, r=2)
                    x1 = qn4[:, :, :, 0:HALF]
                    x2 = qn4[:, :, :, HALF:ROPE]
                    xp = qn4[:, :, :, ROPE:128]

                    def tslice(t):
                        return (t[:, i, :]
                                .rearrange("p (s c) -> p s c", s=2)
                                .unsqueeze(2)
                                .broadcast_to([128, 2, 2, HALF]))

                    t1 = scr.tile([128, 4, HALF], BF16, tag="t1")
                    t2 = scr.tile([128, 4, HALF], BF16, tag="t2")
                    t3 = scr.tile([128, 4, HALF], BF16, tag="t3")
                    t4 = scr.tile([128, 4, HALF], BF16, tag="t4")

                    def v4(t):
                        return t[:].rearrange("p (s r) c -> p s r c", s=2)

                    nc.vector.tensor_mul(v4(t1), x1, tslice(tA_sb))
                    nc.vector.tensor_mul(v4(t2), x2, tslice(tB_sb))
                    nc.vector.tensor_mul(v4(t3), x1, tslice(tC_sb))
                    nc.vector.tensor_mul(v4(t4), x2, tslice(tD_sb))
                    nc.vector.tensor_sub(x1, v4(t1), v4(t2))
                    nc.vector.tensor_add(x2, v4(t3), v4(t4))
                    # passthrough cols [64:128) *= norm weight tail
                    nc.vector.tensor_mul(
                        xp,
                        xp,
                        qk_pass_sb.rearrange("p (s c) -> p s c", s=2)
                        .unsqueeze(2).broadcast_to([128, 2, 2, ROPE]))

                    # collect compressor scores + local V
                    nc.scalar.copy(cwN[:, i:i + 1], ps_l[:, 256:257])
                    nc.vector.tensor_copy(lvn[:, i, :], ps_l[:, 0:256])

                    # transpose q0|q1|lk0|lk1 -> qkT slot i+1 (one batch)
                    ps_t4 = ptp.tile([128, 512], BF16, tag="tp4")
                    for j in range(4):
                        nc.tensor.transpose(
                            ps_t4[:, j * 128:(j + 1) * 128],
                            qn[:, j * 128:(j + 1) * 128], ident_bf[:])
                    nc.scalar.copy(qkT[:, i + 1, :], ps_t4[:])

                    # local-attention branch, software-pipelined into P1
                    for h in range(HPC):
                        cur = (i, h, *front_l(i, h))
                        if pend_l:
                            back_l(*pend_l.pop(0))
                        pend_l.append(cur)
                while pend_l:
                    back_l(*pend_l.pop(0))

                # block-softmax of compressor scores (R=16 blocks)
                ps_cw1 = ptp.tile([16, 128], F32, tag="tp4")
                nc.tensor.transpose(ps_cw1[:], cwN[:], ident_f32[:])
                cwT = scr.tile([16, 128], F32, tag="cwT")
                nc.scalar.copy(cwT[:], ps_cw1[:])
                cwE = scr.tile([16, 128], F32, tag="cwE")
                nc.scalar.activation(cwE[:], cwT[:],
                                     mybir.ActivationFunctionType.Exp)
                csum = sts.tile([16, 8], F32)
                nc.vector.tensor_reduce(
                    csum[:], cwE[:].rearrange("p (g r) -> p g r", g=8),
                    mybir.AxisListType.X, mybir.AluOpType.add)
                crec = sts.tile([16, 8], F32)
                nc.vector.reciprocal(crec[:], csum[:])
                cwW = scr.tile([16, 128], F32, tag="cwW")
                nc.vector.tensor_mul(
                    cwW[:].rearrange("p (g r) -> p g r", g=8),
                    cwE[:].rearrange("p (g r) -> p g r", g=8),
                    crec[:].unsqueeze(2).broadcast_to([16, 8, 16]))
                ps_cw2 = ptp.tile([128, 16], F32, tag="tp4")
                nc.tensor.transpose(ps_cw2[:], cwW[:], ident_f32[0:16, 0:16])
                nc.scalar.copy(wN[:], ps_cw2[:])

            # big const tail + hN pairs (used by P2 on)
            nc.sync.dma_start(out=blob[:, _BIG[0]:_BIG[1]],
                              in_=cblob.ap()[:, _BIG[0]:_BIG[1]])
            hN_tiles = []
            for j in range(NT // 2):
                hN_t = stm.tile([128, 2, HID], BF16, tag="hN", bufs=4)
                nc.sync.dma_start(out=hN_t[:], in_=hN.ap()[j])
                hN_tiles.append(hN_t)

            # ================= P2: entries + ck/cv =================
            with (
                tc.tile_pool(name="ps_e", bufs=1, space="PSUM") as pe,
                tc.tile_pool(name="ps_kv", bufs=1, space="PSUM") as pkv,
                tc.tile_pool(name="ps_tp2", bufs=2, space="PSUM") as ptp2,
            ):
                ps_e = pe.tile([C, HID], F32)
                for j in range(NT // 2):
                    hN_t = hN_tiles[j]
                    for jj in range(2):
                        i = 2 * j + jj
                        wbig = scr.tile([128, 128], BF16, tag="wbig")
                        nc.vector.tensor_scalar_mul(
                            wbig[:], dwide_sb[:, 120 - 8 * i:248 - 8 * i],
                            wN[:, i:i + 1])
                        for hc in range(4):
                            nc.tensor.matmul(
                                ps_e[:, hc * 512:(hc + 1) * 512],
                                wbig[:], hN_t[:, jj, hc * 512:(hc + 1) * 512],
                                start=(i == 0), stop=(i == NT - 1))
                for hc in range(4):
                    nc.vector.tensor_copy(entries[:, hc * 512:(hc + 1) * 512],
                                          ps_e[:, hc * 512:(hc + 1) * 512])
                for kg in range(KT // 4):
                    ps_t = ptp2.tile([128, 4, 128], BF16, tag="tp_e")
                    for kk in range(4):
                        k = kg * 4 + kk
                        nc.tensor.transpose(ps_t[:, kk, :],
                                            entries[:, k * 128:(k + 1) * 128],
                                            ident_bf[:])
                    nc.vector.tensor_copy(eT[:, kg * 4:(kg + 1) * 4, :],
                                          ps_t[:])

                ps_kv = pkv.tile([C, 256], F32)
                for k in range(KT):
                    nc.tensor.matmul(ps_kv[:], eT[:, k, :], wkv_sb[:, k, :],
                                     start=(k == 0),
                                     stop=(zero_bias and k == KT - 1))
                if not zero_bias:
                    nc.tensor.matmul(ps_kv[:], ones1[:], b_kv_sb[:],
                                     start=False, stop=True)

                # ck: rmsnorm + rope at block-end positions
                ssqc = sts.tile([C, 1], F32)
                sq_c = scr.tile([C, 128], F32, tag="sq")
                nc.scalar.activation(sq_c[:], ps_kv[:, 0:128],
                                     mybir.ActivationFunctionType.Square,
                                     accum_out=ssqc[:])
                rmsc = sts.tile([C, 1], F32)
                nc.scalar.activation(rmsc[:], ssqc[:],
                                     mybir.ActivationFunctionType.Sqrt,
                                     scale=1.0 / HD, bias=eps_t[:])
                rinvc = sts.tile([C, 1], F32)
                nc.vector.reciprocal(rinvc[:], rmsc[:])
                ckn = scr.tile([C, 128], F32, tag="ckn")
                nc.vector.tensor_scalar_mul(ckn[:], ps_kv[:, 0:128], rinvc[:])
                ckR = scr.tile([C, 128], BF16, tag="ckR")
                ct1 = scr.tile([C, HALF], F32, tag="ct1")
                ct2 = scr.tile([C, HALF], F32, tag="ct2")
                nc.vector.tensor_mul(ct1[:], ckn[:, 0:HALF], ctA_sb)
                nc.vector.tensor_mul(ct2[:], ckn[:, HALF:ROPE], ctB_sb)
                nc.vector.tensor_sub(ckR[:, 0:HALF], ct1[:], ct2[:])
                nc.vector.tensor_mul(ct1[:], ckn[:, 0:HALF], ctC_sb)
                nc.vector.tensor_mul(ct2[:], ckn[:, HALF:ROPE], ctD_sb)
                nc.vector.tensor_add(ckR[:, HALF:ROPE], ct1[:], ct2[:])
                nc.vector.tensor_mul(ckR[:, ROPE:128], ckn[:, ROPE:128],
                                     ck_pass_sb)
                nc.vector.tensor_copy(cvn[:], ps_kv[:, 128:256])
                ps_ct = ptp2.tile([128, 128], BF16, tag="tp_e")
                nc.tensor.transpose(ps_ct[:], ckR[:], ident_bf[:])
                for h in range(HPC):
                    nc.vector.tensor_copy(ckT_aug[:, h, 0:128], ps_ct[:])

            # ======== P3c+P4: compressed branch + out projection ========
            with (
                tc.tile_pool(name="ps_sc", bufs=3, space="PSUM") as psc,
                tc.tile_pool(name="ps_tpc", bufs=2, space="PSUM") as ptpc,
                tc.tile_pool(name="ps_cc", bufs=2, space="PSUM") as pcc,
                tc.tile_pool(name="ps_out", bufs=1, space="PSUM") as pout,
            ):
                def front_c(i, h):
                    qT = qkT[:, i + 1, h * 128:(h + 1) * 128]
                    ps_s = psc.tile([128, 129], F32, tag="sc")
                    nc.tensor.matmul(ps_s[:], ident_bf[:],
                                     mcmp_sb[:, i, :],
                                     start=True, stop=False)
                    nc.tensor.matmul(ps_s[:], qT, ckT_aug[:, h, :],
                                     start=False, stop=True)
                    p_c = scr.tile([128, 129], BF16, tag="pc", bufs=4)
                    den_c = sts.tile([128, 1], F32)
                    nc.scalar.activation(p_c[:], ps_s[:],
                                         mybir.ActivationFunctionType.Exp,
                                         scale=SCALE, accum_out=den_c[:])
                    return p_c, den_c

                def back_c(i, h, p_c, den_c):
                    rden = sts.tile([128, 1], F32)
                    nc.vector.reciprocal(rden[:], den_c[:])
                    rdg = scr.tile([128, 128], BF16, tag="rdgc", bufs=3)
                    nc.vector.tensor_scalar_mul(rdg[:], ident_bf[:],
                                                rden[:])
                    ps_tpc = ptpc.tile([128, 256], F32, tag="tpc")
                    nc.tensor.matmul(ps_tpc[:, 0:128], p_c[:, 0:128],
                                     rdg[:], start=True, stop=True)
                    nc.tensor.matmul(ps_tpc[0:1, 128:256], p_c[:, 128:129],
                                     rdg[:], start=True, stop=True)
                    pT_c = scr.tile([128, 256], BF16, tag="pTc", bufs=4)
                    if h == 0:
                        nc.vector.tensor_copy(pT_c[:], ps_tpc[:])
                    else:
                        nc.scalar.copy(pT_c[:], ps_tpc[:])
                    ps_cc = pcc.tile([128, 128], F32, tag="cc")
                    nc.tensor.matmul(ps_cc[:], cvn[:], pT_c[:, 0:128],
                                     start=True, stop=False)
                    nc.tensor.matmul(
                        ps_cc[:],
                        sinkv_sb[:, h * 128:(h + 1) * 128],
                        pT_c[0:1, 128:256], start=False, stop=True)
                    # merge into mgT (local ctx already there)
                    mslice = mgT[:, h, i * 128:(i + 1) * 128]
                    nc.vector.tensor_add(mslice, mslice, ps_cc[:])

                def outproj(i):
                    # out projection for s-tile i (0.5 folded into wo)
                    o_sb = scr.tile([128, HID], BF16, tag="o_sb", bufs=2)
                    for q in range(4):
                        ps_o = pout.tile([128, 512], F32, tag="ps_o")
                        for h in range(HPC):
                            nc.tensor.matmul(
                                ps_o[:],
                                mgT[:, h, i * 128:(i + 1) * 128],
                                wo_sb[:, h, q * 512:(q + 1) * 512],
                                start=(h == 0), stop=(h == HPC - 1))
                        dst = o_sb[:, q * 512:(q + 1) * 512]
                        if q % 2 == 0:
                            nc.vector.tensor_copy(dst, ps_o[:])
                        else:
                            nc.scalar.copy(dst, ps_o[:])
                    nc.sync.dma_start(out=out_p.ap()[i], in_=o_sb[:])

                # software pipeline: front(n) ... back(n-1) ... outproj(i)
                pend = None
                for i in range(NT):
                    for h in range(HPC):
                        cur = (i, h, *front_c(i, h))
                        if pend is not None:
                            back_c(*pend)
                            if pend[1] == HPC - 1:
                                outproj(pend[0])
                        pend = cur
                back_c(*pend)
                outproj(pend[0])

    nc.compile()
    return nc


# revision 30
# speedup vs baseline: 1.2006x; 1.1382x over previous
"""Trainium2 Bass kernel for HeavilyCompressedAttention.

Sharding: 16 heads across 8 cores (2 heads/core, tensor-parallel);
compressed-KV path (single shared head) replicated on every core;
out_proj row-parallel with host-side partial sum (bf16 partials).

v2: weight DMAs split per k-group for early P1 start, hN prefetched,
qkT/pT transposes batched through one PSUM tile with single wide
copies, combined per-i mask table, sink handled as rank-1 matmul,
SBUF-only elementwise ops on GpSimd, P4 fused into the P3 loop,
zero-bias fast path.
"""

import os
import sys

import numpy as np
import ml_dtypes

for _p in ("/opt/trn_rl_repo", "/root/.axon_site/_ro/trn_rl_repo"):
    if os.path.isdir(_p) and _p not in sys.path:
        sys.path.insert(0, _p)

from concourse import bacc, mybir  # noqa: E402
import concourse.tile as tile  # noqa: E402
from concourse.bass_utils import run_bass_kernel_spmd  # noqa: E402
from concourse.masks import make_identity  # noqa: E402

F32 = mybir.dt.float32
BF16 = mybir.dt.bfloat16
NPBF = ml_dtypes.bfloat16

S = 2048
HID = 2048
NH = 16
HD = 128
R = 16
C = S // R  # 128
WIN = 128
ROPE = HD // 2  # 64
HALF = ROPE // 2  # 32
EPS = 1e-6
NT = S // 128  # 16 s-tiles
KT = HID // 128  # 16 k-tiles
NCORES = 8
HPC = NH // NCORES  # 2 heads per core
SCALE = 1.0 / float(np.sqrt(HD))
MASKV = -30000.0
KG = 4  # k-tiles per weight-chunk DMA

_CACHE = {}

# ---- const blob column layout (bf16 [128, CB]) ----
_OFF = {}
_cb = 0


def _col(name, n):
    global _cb
    _OFF[name] = (_cb, _cb + n)
    _cb += n


_col("wg0", KG * 769)       # per k: [wqlk_k (512) | wlvc_k (257)]
_col("tA", NT * 64)         # --- SMALL chunk (needed early in P1) ---
_col("tB", NT * 64)
_col("tC", NT * 64)
_col("tD", NT * 64)
_col("qk_pass", 128)
_col("ctA", HALF)           # rows 0:C
_col("ctB", HALF)
_col("ctC", HALF)
_col("ctD", HALF)
_col("ck_pass", ROPE)       # rows 0:C
_col("dwide", 248)
_col("b_qlk", 512)          # row 0
_col("b_lvc", 257)          # row 0
_col("b_kv", 256)           # row 0
_col("sinkkT", HPC)         # [128, HPC] column per head
_col("sinkv", HPC * 128)    # row 0 only used (rank-1 matmul)
for _g in range(1, KT // KG):
    _col(f"wg{_g}", KG * 769)
_col("wkv", KT * 256)       # --- BIG tail (needed from P2 on) ---
_col("wo", HPC * HID)       # [p, h, HID]  (0.5 folded in)
_col("mcomb", NT * 385)     # per-i [local 256 | comp 128 | sink 0] mask
CB = _cb
_SMALL = (_OFF["tA"][0], _OFF["sinkv"][1])
_BIG = (_OFF["wkv"][0], CB)


def _build_bass(zero_bias):
    nc = bacc.Bacc("TRN2", target_bir_lowering=False, debug=False,
                   num_devices=NCORES)

    din = {}

    def inp(name, shape, dt):
        din[name] = nc.dram_tensor(name, list(shape), dt, kind="ExternalInput")
        return din[name]

    hT = inp("hT", [NT, 128, KT * 128], BF16)   # [i][hid_p][k*128+c(s)]
    hN = inp("hN", [NT // 2, 128, 2 * HID], BF16)  # [j][s_p][jj*HID+hid]
    cblob = inp("cblob", [128, CB], BF16)

    out_p = nc.dram_tensor("out_p", [NT, 128, HID], BF16,
                           kind="ExternalOutput")

    with tile.TileContext(nc) as tc:
        with (
            tc.tile_pool(name="const", bufs=1) as cst,
            tc.tile_pool(name="persist", bufs=1) as per,
            tc.tile_pool(name="stream", bufs=3) as stm,
            tc.tile_pool(name="scratch", bufs=3) as scr,
            tc.tile_pool(name="stats", bufs=6) as sts,
        ):
            blob = cst.tile([128, CB], BF16, name="c_blob")
            # wg0 + small tables first (P1 start), wg1..3 next;
            # big tail (wkv|wo|mcomb) emitted just before P2
            a, b = _OFF["wg0"]
            nc.sync.dma_start(out=blob[:, a:b], in_=cblob.ap()[:, a:b])
            nc.sync.dma_start(out=blob[:, _SMALL[0]:_SMALL[1]],
                              in_=cblob.ap()[:, _SMALL[0]:_SMALL[1]])
            for g in range(1, KT // KG):
                a, b = _OFF[f"wg{g}"]
                nc.sync.dma_start(out=blob[:, a:b],
                                  in_=cblob.ap()[:, a:b])

            def cv(name):
                a, b = _OFF[name]
                return blob[:, a:b]

            # weight views: wqlk k -> wg{k//KG} cols [(k%KG)*769, +512]
            def wqlk_k(k):
                base = _OFF[f"wg{k // KG}"][0] + (k % KG) * 769
                return blob[:, base:base + 512]

            def wlvc_k(k):
                base = _OFF[f"wg{k // KG}"][0] + (k % KG) * 769 + 512
                return blob[:, base:base + 257]

            wkv_sb = cv("wkv").rearrange("p (k n) -> p k n", k=KT)
            wo_sb = cv("wo").rearrange("p (h n) -> p h n", h=HPC)
            tA_sb = cv("tA").rearrange("p (i n) -> p i n", i=NT)
            tB_sb = cv("tB").rearrange("p (i n) -> p i n", i=NT)
            tC_sb = cv("tC").rearrange("p (i n) -> p i n", i=NT)
            tD_sb = cv("tD").rearrange("p (i n) -> p i n", i=NT)
            qk_pass_sb = cv("qk_pass")
            ctA_sb = cv("ctA")[0:C, :]
            ctB_sb = cv("ctB")[0:C, :]
            ctC_sb = cv("ctC")[0:C, :]
            ctD_sb = cv("ctD")[0:C, :]
            ck_pass_sb = cv("ck_pass")[0:C, :]
            mcomb_sb = cv("mcomb").rearrange("p (i n) -> p i n", i=NT)
            dwide_sb = cv("dwide")
            b_qlk_sb = cv("b_qlk")[0:1, :]
            b_lvc_sb = cv("b_lvc")[0:1, :]
            b_kv_sb = cv("b_kv")[0:1, :]
            sinkkT_sb = cv("sinkkT")
            sinkv_sb = cv("sinkv")[0:1, :]

            ident_bf = cst.tile([128, 128], BF16)
            make_identity(nc, ident_bf[:])
            ident_f32 = cst.tile([128, 128], F32)
            make_identity(nc, ident_f32[:])
            ones1 = cst.tile([1, 128], BF16)
            nc.vector.memset(ones1[:], 1.0)
            eps_t = cst.tile([128, 1], F32)
            nc.vector.memset(eps_t[:], EPS)

            # ---- persistent activations ----
            lvn = per.tile([128, NT, 256], BF16)    # local v (natural)
            # q0|q1|lk0|lk1 transposed, slot i+1 = tile i; slot0 lk = 0
            qkT = per.tile([128, NT + 1, 512], BF16)
            cwN = per.tile([128, NT], F32)          # compressor scores
            wN = per.tile([128, NT], F32)           # block-softmaxed weights
            entries = per.tile([C, HID], BF16)
            eT = per.tile([128, KT, C], BF16)       # entries^T tiles
            cvn = per.tile([C, 128], BF16)
            ckT_aug = per.tile([128, HPC, 129], BF16)  # ck^T | sink_k col
            mgT = per.tile([128, HPC, S], BF16)     # merged^T = cc+cl

            nc.vector.memset(qkT[:, 0, 256:512], 0.0)
            for h in range(HPC):
                nc.vector.tensor_copy(ckT_aug[:, h, 128:129],
                                      sinkkT_sb[:, h:h + 1])

            # ================= P1: projections + norm/rope =================
            with (
                tc.tile_pool(name="ps_qlk", bufs=2, space="PSUM") as pq,
                tc.tile_pool(name="ps_lvc", bufs=2, space="PSUM") as pl,
                tc.tile_pool(name="ps_tp", bufs=1, space="PSUM") as ptp,
            ):
                for i in range(NT):
                    hT_t = stm.tile([128, KT, 128], BF16, tag="hT")
                    nc.sync.dma_start(out=hT_t[:], in_=hT.ap()[i])
                    ps_q = pq.tile([128, 512], F32)
                    ps_l = pl.tile([128, 257], F32)
                    for k in range(KT):
                        nc.tensor.matmul(ps_q[:], hT_t[:, k, :], wqlk_k(k),
                                         start=(k == 0),
                                         stop=(zero_bias and k == KT - 1))
                        nc.tensor.matmul(ps_l[:], hT_t[:, k, :], wlvc_k(k),
                                         start=(k == 0),
                                         stop=(zero_bias and k == KT - 1))
                    if not zero_bias:
                        nc.tensor.matmul(ps_q[:], ones1[:], b_qlk_sb[:],
                                         start=False, stop=True)
                        nc.tensor.matmul(ps_l[:], ones1[:], b_lvc_sb[:],
                                         start=False, stop=True)

                    # rms norm (over d) for the 4 sub-tensors [q0|q1|k0|k1]
                    ssq = sts.tile([128, 4], F32)
                    for j in range(4):
                        sq_s = scr.tile([128, 128], F32, tag="sq")
                        nc.scalar.activation(
                            sq_s[:], ps_q[:, j * 128:(j + 1) * 128],
                            mybir.ActivationFunctionType.Square,
                            accum_out=ssq[:, j:j + 1])
                    rms = sts.tile([128, 4], F32)
                    nc.scalar.activation(rms[:], ssq[:],
                                         mybir.ActivationFunctionType.Sqrt,
                                         scale=1.0 / HD, bias=eps_t[:])
                    rinv = sts.tile([128, 4], F32)
                    nc.vector.reciprocal(rinv[:], rms[:])

                    qlkn = scr.tile([128, 512], BF16, tag="qlkn", bufs=3)
                    qn = qlkn[:]
                    nc.vector.tensor_mul(
                        qn.rearrange("p (a b) -> p a b", a=4),
                        ps_q[:].rearrange("p (a b) -> p a b", a=4),
                        rinv[:].unsqueeze(2).broadcast_to([128, 4, 128]))
                    # partial rope on cols [0:64) of each sub-tensor;
                    # 4-D views: [p, sect(q/k), head, cols]
                    qn4 = qn.rearrange("p (s r b) -> p s r b", s=2, r=2)
                    x1 = qn4[:, :, :, 0:HALF]
                    x2 = qn4[:, :, :, HALF:ROPE]
                    xp = qn4[:, :, :, ROPE:128]

                    def tslice(t):
                        return (t[:, i, :]
                                .rearrange("p (s c) -> p s c", s=2)
                                .unsqueeze(2)
                                .broadcast_to([128, 2, 2, HALF]))

                    t1 = scr.tile([128, 4, HALF], BF16, tag="t1")
                    t2 = scr.tile([128, 4, HALF], BF16, tag="t2")
                    t3 = scr.tile([128, 4, HALF], BF16, tag="t3")
                    t4 = scr.tile([128, 4, HALF], BF16, tag="t4")

                    def v4(t):
                        return t[:].rearrange("p (s r) c -> p s r c", s=2)

                    nc.vector.tensor_mul(v4(t1), x1, tslice(tA_sb))
                    nc.vector.tensor_mul(v4(t2), x2, tslice(tB_sb))
                    nc.vector.tensor_mul(v4(t3), x1, tslice(tC_sb))
                    nc.vector.tensor_mul(v4(t4), x2, tslice(tD_sb))
                    nc.vector.tensor_sub(x1, v4(t1), v4(t2))
                    nc.vector.tensor_add(x2, v4(t3), v4(t4))
                    # passthrough cols [64:128) *= norm weight tail
                    nc.vector.tensor_mul(
                        xp,
                        xp,
                        qk_pass_sb.rearrange("p (s c) -> p s c", s=2)
                        .unsqueeze(2).broadcast_to([128, 2, 2, ROPE]))

                    # collect compressor scores + local V
                    nc.scalar.copy(cwN[:, i:i + 1], ps_l[:, 256:257])
                    nc.vector.tensor_copy(lvn[:, i, :], ps_l[:, 0:256])

                    # transpose q0|q1|lk0|lk1 -> qkT slot i+1 (one batch)
                    ps_t4 = ptp.tile([128, 512], BF16, tag="tp4")
                    for j in range(4):
                        nc.tensor.transpose(
                            ps_t4[:, j * 128:(j + 1) * 128],
                            qn[:, j * 128:(j + 1) * 128], ident_bf[:])
                    nc.scalar.copy(qkT[:, i + 1, :], ps_t4[:])

                # block-softmax of compressor scores (R=16 blocks)
                ps_cw1 = ptp.tile([16, 128], F32, tag="tp_cw1")
                nc.tensor.transpose(ps_cw1[:], cwN[:], ident_f32[:])
                cwT = scr.tile([16, 128], F32, tag="cwT")
                nc.scalar.copy(cwT[:], ps_cw1[:])
                cwE = scr.tile([16, 128], F32, tag="cwE")
                nc.scalar.activation(cwE[:], cwT[:],
                                     mybir.ActivationFunctionType.Exp)
                csum = sts.tile([16, 8], F32)
                nc.vector.tensor_reduce(
                    csum[:], cwE[:].rearrange("p (g r) -> p g r", g=8),
                    mybir.AxisListType.X, mybir.AluOpType.add)
                crec = sts.tile([16, 8], F32)
                nc.vector.reciprocal(crec[:], csum[:])
                cwW = scr.tile([16, 128], F32, tag="cwW")
                nc.vector.tensor_mul(
                    cwW[:].rearrange("p (g r) -> p g r", g=8),
                    cwE[:].rearrange("p (g r) -> p g r", g=8),
                    crec[:].unsqueeze(2).broadcast_to([16, 8, 16]))
                ps_cw2 = ptp.tile([128, 16], F32, tag="tp_cw2")
                nc.tensor.transpose(ps_cw2[:], cwW[:], ident_f32[0:16, 0:16])
                nc.scalar.copy(wN[:], ps_cw2[:])

            # big const tail + hN pairs (used by P2 on)
            nc.sync.dma_start(out=blob[:, _BIG[0]:_BIG[1]],
                              in_=cblob.ap()[:, _BIG[0]:_BIG[1]])
            hN_tiles = []
            for j in range(NT // 2):
                hN_t = stm.tile([128, 2, HID], BF16, tag="hN", bufs=4)
                nc.sync.dma_start(out=hN_t[:], in_=hN.ap()[j])
                hN_tiles.append(hN_t)

            # ================= P2: entries + ck/cv =================
            with (
                tc.tile_pool(name="ps_e", bufs=1, space="PSUM") as pe,
                tc.tile_pool(name="ps_kv", bufs=1, space="PSUM") as pkv,
                tc.tile_pool(name="ps_tp2", bufs=2, space="PSUM") as ptp2,
            ):
                ps_e = pe.tile([C, HID], F32)
                for j in range(NT // 2):
                    hN_t = hN_tiles[j]
                    for jj in range(2):
                        i = 2 * j + jj
                        wbig = scr.tile([128, 128], BF16, tag="wbig")
                        nc.vector.tensor_scalar_mul(
                            wbig[:], dwide_sb[:, 120 - 8 * i:248 - 8 * i],
                            wN[:, i:i + 1])
                        for hc in range(4):
                            nc.tensor.matmul(
                                ps_e[:, hc * 512:(hc + 1) * 512],
                                wbig[:], hN_t[:, jj, hc * 512:(hc + 1) * 512],
                                start=(i == 0), stop=(i == NT - 1))
                for hc in range(4):
                    nc.vector.tensor_copy(entries[:, hc * 512:(hc + 1) * 512],
                                          ps_e[:, hc * 512:(hc + 1) * 512])
                for kg in range(KT // 4):
                    ps_t = ptp2.tile([128, 4, 128], BF16, tag="tp_e")
                    for kk in range(4):
                        k = kg * 4 + kk
                        nc.tensor.transpose(ps_t[:, kk, :],
                                            entries[:, k * 128:(k + 1) * 128],
                                            ident_bf[:])
                    nc.vector.tensor_copy(eT[:, kg * 4:(kg + 1) * 4, :],
                                          ps_t[:])

                ps_kv = pkv.tile([C, 256], F32)
                for k in range(KT):
                    nc.tensor.matmul(ps_kv[:], eT[:, k, :], wkv_sb[:, k, :],
                                     start=(k == 0),
                                     stop=(zero_bias and k == KT - 1))
                if not zero_bias:
                    nc.tensor.matmul(ps_kv[:], ones1[:], b_kv_sb[:],
                                     start=False, stop=True)

                # ck: rmsnorm + rope at block-end positions
                ssqc = sts.tile([C, 1], F32)
                sq_c = scr.tile([C, 128], F32, tag="sq")
                nc.scalar.activation(sq_c[:], ps_kv[:, 0:128],
                                     mybir.ActivationFunctionType.Square,
                                     accum_out=ssqc[:])
                rmsc = sts.tile([C, 1], F32)
                nc.scalar.activation(rmsc[:], ssqc[:],
                                     mybir.ActivationFunctionType.Sqrt,
                                     scale=1.0 / HD, bias=eps_t[:])
                rinvc = sts.tile([C, 1], F32)
                nc.vector.reciprocal(rinvc[:], rmsc[:])
                ckn = scr.tile([C, 128], F32, tag="ckn")
                nc.vector.tensor_scalar_mul(ckn[:], ps_kv[:, 0:128], rinvc[:])
                ckR = scr.tile([C, 128], BF16, tag="ckR")
                ct1 = scr.tile([C, HALF], F32, tag="ct1")
                ct2 = scr.tile([C, HALF], F32, tag="ct2")
                nc.vector.tensor_mul(ct1[:], ckn[:, 0:HALF], ctA_sb)
                nc.vector.tensor_mul(ct2[:], ckn[:, HALF:ROPE], ctB_sb)
                nc.vector.tensor_sub(ckR[:, 0:HALF], ct1[:], ct2[:])
                nc.vector.tensor_mul(ct1[:], ckn[:, 0:HALF], ctC_sb)
                nc.vector.tensor_mul(ct2[:], ckn[:, HALF:ROPE], ctD_sb)
                nc.vector.tensor_add(ckR[:, HALF:ROPE], ct1[:], ct2[:])
                nc.vector.tensor_mul(ckR[:, ROPE:128], ckn[:, ROPE:128],
                                     ck_pass_sb)
                nc.vector.tensor_copy(cvn[:], ps_kv[:, 128:256])
                ps_ct = ptp2.tile([128, 128], BF16, tag="tp_e")
                nc.tensor.transpose(ps_ct[:], ckR[:], ident_bf[:])
                for h in range(HPC):
                    nc.vector.tensor_copy(ckT_aug[:, h, 0:128], ps_ct[:])

            # ============ P3+P4: attention + out projection ============
            with (
                tc.tile_pool(name="ps_sc", bufs=2, space="PSUM") as psc,
                tc.tile_pool(name="ps_ctx", bufs=2, space="PSUM") as pcx,
                tc.tile_pool(name="ps_tp3", bufs=2, space="PSUM") as ptp3,
                tc.tile_pool(name="ps_out", bufs=1, space="PSUM") as pout,
            ):
                def front(i, h):
                    qT = qkT[:, i + 1, h * 128:(h + 1) * 128]
                    ps_s = psc.tile([128, 385], F32, tag="ps_s")
                    # mask pre-loaded into psum via PE (ident.T @ mcomb_i),
                    # scores accumulate on top
                    nc.tensor.matmul(ps_s[:], ident_bf[:],
                                     mcomb_sb[:, i, :],
                                     start=True, stop=False)
                    nc.tensor.matmul(
                        ps_s[:, 0:256], qT,
                        qkT[:, i:i + 2, 256 + h * 128:256 + (h + 1) * 128],
                        start=False, stop=True)
                    nc.tensor.matmul(ps_s[:, 256:385], qT,
                                     ckT_aug[:, h, :],
                                     start=False, stop=True)
                    # two separate softmaxes: local (cols 0:256) and
                    # compressed+sink (cols 256:385)
                    p_t = scr.tile([128, 385], BF16, tag="p", bufs=4)
                    den = sts.tile([128, 2], F32)
                    nc.scalar.activation(p_t[:, 0:256], ps_s[:, 0:256],
                                         mybir.ActivationFunctionType.Exp,
                                         scale=SCALE,
                                         accum_out=den[:, 0:1])
                    nc.scalar.activation(p_t[:, 256:385], ps_s[:, 256:385],
                                         mybir.ActivationFunctionType.Exp,
                                         scale=SCALE,
                                         accum_out=den[:, 1:2])
                    return p_t, den

                def back(i, h, p_t, den):
                    rden = sts.tile([128, 2], F32)
                    nc.vector.reciprocal(rden[:], den[:])
                    # normalization fused into the p-transposes:
                    # matmul(out, x, diag(r)) = x.T @ diag(r)
                    rdg = scr.tile([128, 256], BF16, tag="rdg", bufs=3)
                    nc.vector.tensor_scalar_mul(rdg[:, 0:128],
                                                ident_bf[:],
                                                rden[:, 0:1])
                    nc.vector.tensor_scalar_mul(rdg[:, 128:256],
                                                ident_bf[:],
                                                rden[:, 1:2])
                    ps_tp_t = ptp3.tile([128, 512], F32, tag="tp3")
                    for cidx in range(3):
                        nc.tensor.matmul(
                            ps_tp_t[:, cidx * 128:(cidx + 1) * 128],
                            p_t[:, cidx * 128:(cidx + 1) * 128],
                            rdg[:, 0:128] if cidx < 2 else rdg[:, 128:256],
                            start=True, stop=True)
                    nc.tensor.matmul(ps_tp_t[0:1, 384:512],
                                     p_t[:, 384:385], rdg[:, 128:256],
                                     start=True, stop=True)
                    pT_sb = scr.tile([128, 512], BF16, tag="pT", bufs=4)
                    if h == 0:
                        nc.vector.tensor_copy(pT_sb[:], ps_tp_t[:])
                    else:
                        nc.scalar.copy(pT_sb[:], ps_tp_t[:])
                    pT_sink = pT_sb[0:1, 384:512]

                    ps_c = pcx.tile([128, 128], F32, tag="ps_c")
                    nc.tensor.matmul(ps_c[:],
                                     lvn[:, i - 1 if i else NT - 1,
                                         h * 128:(h + 1) * 128],
                                     pT_sb[:, 0:128],
                                     start=True, stop=False)
                    nc.tensor.matmul(ps_c[:],
                                     lvn[:, i, h * 128:(h + 1) * 128],
                                     pT_sb[:, 128:256],
                                     start=False, stop=False)
                    nc.tensor.matmul(ps_c[:], cvn[:], pT_sb[:, 256:384],
                                     start=False, stop=False)
                    nc.tensor.matmul(
                        ps_c[:],
                        sinkv_sb[:, h * 128:(h + 1) * 128],
                        pT_sink, start=False, stop=True)
                    nc.vector.tensor_copy(
                        mgT[:, h, i * 128:(i + 1) * 128], ps_c[:])

                def outproj(i):
                    # out projection for s-tile i (0.5 folded into wo)
                    o_sb = scr.tile([128, HID], BF16, tag="o_sb", bufs=2)
                    for half in range(2):
                        ps_o = pout.tile([128, HID // 2], F32, tag="ps_o")
                        for h in range(HPC):
                            for oc in range(2):
                                occ = half * 2 + oc
                                nc.tensor.matmul(
                                    ps_o[:, oc * 512:(oc + 1) * 512],
                                    mgT[:, h, i * 128:(i + 1) * 128],
                                    wo_sb[:, h, occ * 512:(occ + 1) * 512],
                                    start=(h == 0), stop=(h == HPC - 1))
                        dst = o_sb[:, half * 1024:(half + 1) * 1024]
                        if half == 0:
                            nc.vector.tensor_copy(dst, ps_o[:])
                        else:
                            nc.scalar.copy(dst, ps_o[:])
                    nc.sync.dma_start(out=out_p.ap()[i], in_=o_sb[:])

                # software pipeline: front(n) ... back(n-1) ... outproj(i)
                pend = None
                for i in range(NT):
                    for h in range(HPC):
                        cur = (i, h, *front(i, h))
                        if pend is not None:
                            back(*pend)
                            if pend[1] == HPC - 1:
                                outproj(pend[0])
                        pend = cur
                back(*pend)
                outproj(pend[0])

    nc.compile()
    return nc
